# revision 23
# baseline (speedup 1.0000x reference)
"""Bidirectional attention (softmax(+logits) and softmax(-logits) branches)
on 8 Trainium2 NeuronCores.

Sharding: batch x head-group. Core c handles batch c//4 and heads
4*(c%4) .. 4*(c%4)+3. Each core computes its heads' Q/K/V projections,
both softmax branches, and a partial output projection (row-shard of Wo);
the host sums the 4 partials per batch and transposes.

All matmuls run in bf16 (fp32 matmul is 4x slower on the PE); PSUM
accumulation is fp32. The softmax uses unnormalized exp (logit range is
~N(0,1), no max-subtraction needed): P_pos = exp(+logits) via wide ACT
ops; P_neg = 1/P_pos via a single custom-DVE op (BITWISE_NOT seed + 2
inline Newton passes) run on the bf16 tiles directly -- this halves the
ACT engine's exp volume. Z comes free from a ones-column appended to V;
the 1/Z normalization is batched across both branches.

Pipeline structure (all HW-measured):
- ONE unified PSUM allocation: lgp (2x[128,1024]) + ftp (2x[128,1024])
  = 8 banks, live for the whole kernel.  Phases 1/3 borrow tiles from
  these rotations instead of opening their own pools (separate pools
  alias banks and serialize phase boundaries).  Matmul psum outputs
  must be BANK-aligned (mid-bank outputs corrupt on HW).
- Head-0 PRELUDE: the first head's logits/exp/recip are emitted before
  the V projection with its PV matmuls deferred, so the ACT exp stream
  starts as soon as Q/K are projected (~25us) instead of after the
  whole V projection (~50us).  The PV backlog drains 2-groups-per-
  iteration during head 1.
- PV matmuls trail their logits: pos by 1 group, neg by 2.  The serial
  chains logits -> exp(ACT) -> PV-pos (~1.1us) and -> recip(DVE) ->
  PV-neg (~2.2us) would otherwise stall the in-order PE queue every
  group.  PSUM accumulation order within a region is free; only the
  start (st==0) / stop (st==SK-1) flags matter.
- The ACT-computed neg-branch exps sit at g==3/4 (mid-head), away from
  the head-boundary congestion.
- Output projection (phase 3) alternates its borrowed psum pair between
  the lg and ft rotations (4 tiles in flight) and splits each evac into
  concurrent DVE+ACT halves, so it runs matmul-paced.
- full-width head-pair projections: one P_out=128 matmul covers both
  heads of a pair; Q^T/K^T kept in base-0 and base-64 copies so
  consecutive K=64 logit matmuls alternate PE row-groups.

Host-side prep folds the 1/sqrt(d) scale into Wq, and bv@Wo+bo into a
host-side bias (exact because softmax rows sum to 1).
"""

import os
import sys

for _p in ("/opt/trn_rl_repo",):
    if _p not in sys.path:
        sys.path.insert(0, _p)

import numpy as np
import ml_dtypes

import concourse.bass as bass
import concourse.tile as tile
from concourse import bacc, mybir
from concourse import bass_utils
from concourse.dve_ops import RECIPROCAL_APPROX_FAST, RECIP_APPROX_FAST_CONSTS

BF16 = ml_dtypes.bfloat16

B, S, D = 2, 2048, 1024
NUM_HEADS, HEAD_DIM = 16, 64
N_CORES = 8
GROUPS = 4                      # head groups (one per core within a batch)
H = NUM_HEADS // GROUPS         # heads per core = 4
DH = H * HEAD_DIM               # per-core head dims = 256
QCH = 512                       # q-chunk (matmul moving free dim)
NCH = S // QCH                  # 4 q-chunks
SK = S // 128                   # 16 s_k tiles
VSTRIDE = H * 128               # V' row stride: 4 heads x (64 v + 64 ones)

f32 = mybir.dt.float32
bf16 = mybir.dt.bfloat16
EXP = mybir.ActivationFunctionType.Exp
IDENT = mybir.ActivationFunctionType.Identity

# How many groups the PV matmuls trail their group's logits.
POS_DELAY = 1
NEG_DELAY = 2
# Max deferred-PV groups flushed per ring iteration (drains the head-0
# prelude backlog without monopolizing the PE).
FLUSH_CAP = 2

_CACHE = {}


def _emit(tc):
    nc = tc.nc
    tokT = nc.dram_tensor("tokT", [D, S], bf16, kind="ExternalInput").ap()
    wq = nc.dram_tensor("wq", [128, 8 * DH], bf16, kind="ExternalInput").ap()
    wk = nc.dram_tensor("wk", [128, 8 * DH], bf16, kind="ExternalInput").ap()
    wv = nc.dram_tensor("wv", [128, 8 * DH], bf16, kind="ExternalInput").ap()
    wo = nc.dram_tensor("wo", [128, 2 * D], bf16, kind="ExternalInput").ap()
    bqk = nc.dram_tensor("bqk", [128, 4], f32, kind="ExternalInput").ap()
    outs = [
        nc.dram_tensor("outT_neg", [D, S], bf16, kind="ExternalOutput").ap(),
        nc.dram_tensor("outT_pos", [D, S], bf16, kind="ExternalOutput").ap(),
    ]

    rc = RECIP_APPROX_FAST_CONSTS

    import contextlib

    with contextlib.ExitStack() as ctx:
        wp = ctx.enter_context(tc.tile_pool(name="wp", bufs=1))
        act = ctx.enter_context(tc.tile_pool(name="act", bufs=1))
        ppool = ctx.enter_context(tc.tile_pool(name="pp", bufs=2))
        fpool = ctx.enter_context(tc.tile_pool(name="fp", bufs=4))
        zpool = ctx.enter_context(tc.tile_pool(name="zp", bufs=2))
        opool = ctx.enter_context(tc.tile_pool(name="op", bufs=8))
        # the single 8-bank PSUM allocation for the whole kernel
        lgp = ctx.enter_context(tc.tile_pool(name="lgp", bufs=2, space="PSUM"))
        ftp = ctx.enter_context(tc.tile_pool(name="ftp", bufs=2, space="PSUM"))

        # ---- weight / bias / token loads -------------------------------
        # input loads in COMPUTE order: the K projection's t-chain chases
        # tok tiles arriving on sync, while the other weights ride the
        # gpsimd/scalar queues (HBM bandwidth is the real floor, so the
        # first-needed data must not share it with late-needed data)
        wq_sb = wp.tile([128, 8 * DH], bf16, tag="wq")
        wk_sb = wp.tile([128, 8 * DH], bf16, tag="wk")
        wv_sb = wp.tile([128, 8 * DH], bf16, tag="wv")
        wo_sb = wp.tile([128, 2 * D], bf16, tag="wo")
        bqk_sb = wp.tile([128, 4], f32, tag="bqk")
        # everything rides ONE queue in strict need-order so the
        # first-needed bytes get the full HBM bandwidth: wk+wq (the
        # concurrent K/Q t-chains), tok (their pacer), wv (V starts
        # after the Q evacs), wo (needed only at phase 3)
        nc.sync.dma_start(wk_sb[:], wk)
        nc.sync.dma_start(wq_sb[:], wq)
        nc.gpsimd.dma_start(bqk_sb[:], bqk)

        tok = []
        for t in range(8):
            tt = act.tile([128, S], bf16, tag=f"tok{t}")
            nc.sync.dma_start(tt[:], tokT[t * 128:(t + 1) * 128, :])
            tok.append(tt)
        nc.sync.dma_start(wv_sb[:], wv)
        nc.sync.dma_start(wo_sb[:], wo)

        # head-pair tiles: even head in partitions 0:64, odd in 64:128;
        # *_alt has the two halves swapped (so every head exists at both
        # partition bases -- lets logit matmuls alternate PE row groups)
        qt_pair = [act.tile([128, S], bf16, tag=f"qp{j}", name=f"qp{j}") for j in range(2)]
        kt_pair = [act.tile([128, S], bf16, tag=f"kp{j}", name=f"kp{j}") for j in range(2)]
        qt_alt = [act.tile([128, S], bf16, tag=f"qa{j}", name=f"qa{j}") for j in range(2)]
        kt_alt = [act.tile([128, S], bf16, tag=f"ka{j}", name=f"ka{j}") for j in range(2)]
        SKQ = SK // 4
        vp_q = [act.tile([128, SKQ * VSTRIDE], bf16, tag=f"vp{i}", name=f"vp{i}")
                for i in range(4)]

        def vap_for(st, h):
            q, r = divmod(st, SKQ)
            off = r * VSTRIDE + h * 128
            return vp_q[q][:, off:off + 128]

        # V' head block is [ones (64) | v (64)]: the 64 ones-columns make
        # the PV matmul write Z replicated across psum rows 0:64 -- a free
        # partition-broadcast of the softmax denominator (and a full
        # 128-col stationary enables FWL).  Z sits at base partition 0
        # because the custom-DVE reciprocal only works at base 0
        # (HW-verified); feat lands at rows 64:128.
        for q in range(4):
            ones_ap = vp_q[q].rearrange(
                "p (s h x) -> p (s h) x", s=SKQ, h=H)[:, :, 0:64]
            nc.gpsimd.memset(ones_ap, 1.0)

        def lgp_chains():
            tiles = [lgp.tile([128, GW * QCH], f32, tag="lg",
                              name=f"pjb{x}") for x in range(2)]
            return [tiles[x // 2][:, (x % 2) * QCH:(x % 2 + 1) * QCH]
                    for x in range(4)]

        def ftp_chains():
            tiles = [ftp.tile([128, GW * QCH], f32, tag="ft",
                              name="ft") for x in range(2)]
            return [tiles[x // 2][:, (x % 2) * QCH:(x % 2 + 1) * QCH]
                    for x in range(4)]

        # ---- phase 1: projections (borrowed ring psum tiles) -----------
        # (matmul psum outputs are capped at one bank = 512 f32 columns,
        # and must be bank-aligned)
        def qk_evac(j, pair, alt, ps, bcol0, on_dve):
            # evacs split across ACT (identity+bias) and DVE
            # (tensor_scalar add) so 8 of them don't serialize one engine
            for cc in range(4):
                dst = pair[j][:, cc * QCH:(cc + 1) * QCH]
                bias_ap = bqk_sb[:, bcol0 + j:bcol0 + j + 1]
                if on_dve:
                    nc.vector.tensor_scalar_add(dst, ps[cc], bias_ap)
                else:
                    nc.scalar.activation(dst, ps[cc], IDENT, bias=bias_ap)
            nc.sync.dma_start(alt[j][64:128, :], pair[j][0:64, :])
            nc.sync.dma_start(alt[j][0:64, :], pair[j][64:128, :])

        def qk_chain_t(ps, w_sb, base, t, first, last):
            for cc in range(4):
                nc.tensor.matmul(ps[cc], w_sb[:, base:base + 128],
                                 tok[t][:, cc * QCH:(cc + 1) * QCH],
                                 start=first, stop=last)

        def qk_proj(j):
            # full-width head-pair matmuls: one P_out=128 matmul per
            # (din-tile, chunk-pair) covers both heads (even head -> psum
            # rows 0:64, odd -> 64:128 -- weight cols are adjacent).
            # Pair 0 runs K and Q as CONCURRENT t-chains (K on the lg
            # psum pair, Q on the ft pair) so both finish with the token
            # DMA instead of Q serializing behind K's evac.  Pair 1 runs
            # mid-ring where the ft rotation is busy, so its Q chains
            # reuse the lg pair sequentially.
            base_j = j * 128
            if j == 0:
                ps_k = lgp_chains()
                ps_q = ftp_chains()
                for t in range(8):
                    base = t * DH + base_j
                    qk_chain_t(ps_k, wk_sb, base, t, t == 0, t == 7)
                    qk_chain_t(ps_q, wq_sb, base, t, t == 0, t == 7)
                qk_evac(j, kt_pair, kt_alt, ps_k, 2, on_dve=False)
                qk_evac(j, qt_pair, qt_alt, ps_q, 0, on_dve=True)
            else:
                for w_sb, pair, alt, bcol0, on_dve in (
                    (wk_sb, kt_pair, kt_alt, 2, False),
                    (wq_sb, qt_pair, qt_alt, 0, True),
                ):
                    ps = lgp_chains()
                    for t in range(8):
                        base = t * DH + base_j
                        qk_chain_t(ps, w_sb, base, t, t == 0, t == 7)
                    qk_evac(j, pair, alt, ps, bcol0, on_dve)

        def emit_vproj():
            # V: 2 s-tile chains per sp-group, carved from one borrowed
            # ftp tile at BANK-ALIGNED offsets (0 and QCH*4B=2KB).
            # Matmul psum outputs at mid-bank offsets corrupt on HW, so
            # each [128,256] chain gets its own bank (half unused).
            for sp in range(SK // 2):
                ftv = ftp.tile([128, 2 * QCH], f32, tag="ft", name="ft")
                psv = [ftv[:, i * QCH:i * QCH + DH] for i in range(2)]
                for t in range(8):
                    for i in range(2):
                        st = sp * 2 + i
                        nc.tensor.matmul(
                            psv[i],
                            tok[t][:, st * 128:(st + 1) * 128],
                            wv_sb[:, t * DH:(t + 1) * DH],
                            start=(t == 0), stop=(t == 7),
                        )
                for i in range(2):
                    st = sp * 2 + i
                    q, r = divmod(st, SKQ)
                    dst = vp_q[q][:, r * VSTRIDE:(r + 1) * VSTRIDE]
                    dst3 = dst.rearrange("p (h x) -> p h x", h=H)[:, :, 64:128]
                    src3 = psv[i].rearrange("p (h x) -> p h x", h=H)
                    nc.vector.tensor_copy(dst3, src3)

        GW = 2           # s_k-tiles per exp group (FD = GW*QCH = 1024)
        NG = SK // GW    # 8 groups per (head, chunk)

        qk_proj(0)
        # (the V projection and pair-1 q/k are emitted at (c0,h1,g0),
        # after head 0's logits/exp prelude: the PE chews them while the
        # ACT/DVE stream is already running)

        # ---- phase 2: attention ----------------------------------------
        fsb_all = []
        # phase-3 output DMAs rotate over the three DMA-capable queues
        out_engines = [nc.sync, nc.gpsimd, nc.scalar]
        u_out = [0]

        # per-(c,h) feature psum, allocated lazily at the first deferred
        # PV flush so the ft rotation order matches actual use
        ft_map = {}

        def get_ft(c2, h2):
            if (c2, h2) not in ft_map:
                ft_map[(c2, h2)] = ftp.tile([128, 2 * QCH], f32, tag="ft",
                                            name="ft")
            return ft_map[(c2, h2)]

        def outproj_dp(c, dp):
            # one dout-pair iteration of the output projection: a psum
            # pair borrowed alternately from the lg / ft rotations (so 4
            # tiles pipeline), 2 dh-passes x 2 branches accumulation,
            # evac split into concurrent DVE (neg) + ACT (pos) halves,
            # output DMAs on sync/gpsimd
            pool, tg = (lgp, "lg") if dp % 2 == 0 else (ftp, "ft")
            ops = [pool.tile([128, GW * QCH], f32, tag=tg, name=f"ops{i}")
                   for i in range(2)]
            for p in range(2):
                for i in range(2):
                    dt = dp * 2 + i
                    lhs = wo_sb[:, p * D + dt * 128:p * D + (dt + 1) * 128]
                    for br in range(2):
                        nc.tensor.matmul(
                            ops[i][:, br * QCH:(br + 1) * QCH],
                            lhs,
                            fsb_all[c][p][:, br * QCH:(br + 1) * QCH],
                            start=(p == 0), stop=(p == 1),
                        )
            for i in range(2):
                dt = dp * 2 + i
                osb = opool.tile([128, 2 * QCH], bf16, tag="os", name="osb")
                nc.vector.tensor_copy(osb[:, 0:QCH], ops[i][:, 0:QCH])
                nc.scalar.copy(osb[:, QCH:2 * QCH], ops[i][:, QCH:2 * QCH])
                for br in range(2):
                    eng = out_engines[u_out[0] % len(out_engines)]
                    u_out[0] += 1
                    eng.dma_start(
                        outs[br][dt * 128:(dt + 1) * 128,
                                 c * QCH:(c + 1) * QCH],
                        osb[:, br * QCH:(br + 1) * QCH],
                    )

        # deferred normalization: the recip -> zhi-DMA -> mul chain of
        # head h is emitted split across head h+1's groups 3/5 (after
        # head h's delayed PV matmuls have all been flushed), so the
        # sync-DMA round trip overlaps queued recip work instead of
        # stalling the strict-FIFO DVE queue at each head boundary
        def norm_start(stt):
            c2, h2, jp, parp, fsbp = stt
            ft2p = ft_map[(c2, h2)]
            zb = zpool.tile([64, 2 * QCH], f32, tag="zb", name="zb")
            zhi = zpool.tile([128, 2 * QCH], f32, tag="zhi", name="zhi")
            nc.vector.reciprocal_approx_fast(zb[:], ft2p[0:64, :])
            nc.sync.dma_start(zhi[64:128, :], zb[:])
            return (ft2p, parp, fsbp, zhi)

        def norm_finish(stt):
            ft2p, parp, fsbp, zhi = stt
            if parp == 1:
                # odd head: feat rows 64:128 align with the fsb
                # destination -- write it directly
                nc.vector.tensor_mul(fsbp[64:128, :],
                                     ft2p[64:128, :], zhi[64:128, :])
            else:
                tmp = zpool.tile([128, 2 * QCH], bf16, tag="tmp",
                                 name="tmp")
                nc.vector.tensor_mul(tmp[64:128, :], ft2p[64:128, :],
                                     zhi[64:128, :])
                nc.sync.dma_start(fsbp[0:64, :], tmp[64:128, :])

        def emit_logits(c2, h2, g2):
            # K=64 logit matmul pair on alternating PE row groups
            j2, par2 = h2 // 2, h2 % 2
            k_lo = kt_pair[j2] if par2 == 0 else kt_alt[j2]
            k_hi = kt_alt[j2] if par2 == 0 else kt_pair[j2]
            q_lo = qt_pair[j2] if par2 == 0 else qt_alt[j2]
            q_hi = qt_alt[j2] if par2 == 0 else qt_pair[j2]
            lg = lgp.tile([128, GW * QCH], f32, tag="lg")
            for t2 in range(GW):
                st = g2 * GW + t2
                if st % 2 == 0:
                    nc.tensor.matmul(
                        lg[:, t2 * QCH:(t2 + 1) * QCH],
                        k_lo[0:64, st * 128:(st + 1) * 128],
                        q_lo[0:64, c2 * QCH:(c2 + 1) * QCH],
                        start=True, stop=True)
                else:
                    nc.tensor.matmul(
                        lg[:, t2 * QCH:(t2 + 1) * QCH],
                        k_hi[64:128, st * 128:(st + 1) * 128],
                        q_hi[64:128, c2 * QCH:(c2 + 1) * QCH],
                        start=True, stop=True)
            return lg

        def emit_pv(item, br):
            pw, c2, h2, g2 = item
            ft2_ = get_ft(c2, h2)
            for t2 in range(GW):
                st = g2 * GW + t2
                nc.tensor.matmul(
                    ft2_[:, br * QCH:(br + 1) * QCH],
                    vap_for(st, h2),
                    pw[:, t2 * QCH:(t2 + 1) * QCH],
                    start=(st == 0),
                    stop=(st == SK - 1),
                )

        # flattened group sequence.  Logits are software-pipelined ONE
        # group ahead; PV-pos trails POS_DELAY and PV-neg NEG_DELAY
        # groups so the ACT/DVE chains are covered by other PE work.
        # Head (c0,h0) is a PRELUDE: its PV is fully deferred past the
        # V projection.
        groups = [(c, h, g) for c in range(NCH) for h in range(H)
                  for g in range(NG)]
        lg_next = None
        pend = pend2 = None
        pend_pos = []
        pend_neg = []

        for i, (c, h, g) in enumerate(groups):
            j, par = h // 2, h % 2
            if g == 0 and h == 0:
                fsb = [fpool.tile([128, 2 * QCH], bf16, tag="fsb",
                                  bufs=8, name=f"fsb{j2}")
                       for j2 in range(2)]
                fsb_all.append(fsb)
                fsb_c = fsb
            if c == 0 and h == 1 and g == 0:
                # the deferred phase-1 blocks: V projection (8 ft-tile
                # sp-groups) and pair-1 q/k (4 lg tiles), emitted under
                # head 0's already-running exp/recip stream
                emit_vproj()
                qk_proj(1)
            if i == 0:
                lg_next = emit_logits(c, h, g)
            lg = lg_next
            if i + 1 < len(groups):
                lg_next = emit_logits(*groups[i + 1])
            pw_pos = ppool.tile([128, GW * QCH], bf16, tag="pwp",
                                name="pwp", bufs=11)
            pw_neg = ppool.tile([128, GW * QCH], bf16, tag="pwn",
                                name="pwn", bufs=12)
            nc.scalar.activation(pw_pos[:], lg[:], EXP)
            if g == 3 or (g == 4 and ((h + c) & 1)):
                # mid-head ACT-exp groups balance ACT vs DVE without
                # piling a second exp onto the busy head-boundary groups
                nc.scalar.activation(pw_neg[:], lg[:], EXP, scale=-1.0)
            else:
                # P_neg = 1/P_pos on DVE (bf16 in/out; the DVE read
                # path converts bf16->fp32 bit-exactly so the fp32
                # BITWISE_NOT seed applies unchanged)
                nc.vector._custom_dve(
                    RECIPROCAL_APPROX_FAST,
                    out=pw_neg[:], in0=pw_pos[:],
                    s0=rc["s0"], s1=rc["s1"], imm2=rc["imm2"],
                )
            if not (c == 0 and h == 0):
                # flush deferred PV: neg first (its recip is oldest)
                for _ in range(FLUSH_CAP):
                    if len(pend_neg) > NEG_DELAY:
                        emit_pv(pend_neg.pop(0), 0)
                for _ in range(FLUSH_CAP):
                    if len(pend_pos) > POS_DELAY:
                        emit_pv(pend_pos.pop(0), 1)
            pend_pos.append((pw_pos, c, h, g))
            pend_neg.append((pw_neg, c, h, g))
            # previous head's deferred normalization: feat / Z.  Z sits
            # pre-broadcast in its psum rows 0:64 (the ones-columns),
            # feat in rows 64:128, both branches in one [64, 2*QCH] op.
            # recip at base 0 (the custom-DVE op requires it), one SBUF
            # DMA shifts 1/Z up to rows 64:128, multiply straight out of
            # PSUM.
            if g == 3 and pend is not None:
                pend2, pend = norm_start(pend), None
            elif g == 5 and pend2 is not None:
                norm_finish(pend2)
                pend2 = None
            if g == NG - 1:
                pend = (c, h, j, par, fsb_c[j])

        while pend_pos or pend_neg:
            if pend_neg:
                emit_pv(pend_neg.pop(0), 0)
            if pend_pos:
                emit_pv(pend_pos.pop(0), 1)
        norm_finish(norm_start(pend))

        # ---- phase 3: output projection (borrowed lg/ft psum) ----------
        for c in range(NCH):
            for dp in range(4):
                outproj_dp(c, dp)


def _build():
    if "nc" in _CACHE:
        return _CACHE["nc"]
    nc = bacc.Bacc("TRN2", target_bir_lowering=False, debug=False,
                   num_devices=N_CORES)
    with tile.TileContext(nc) as tc:
        _emit(tc)
    nc.compile()
    _CACHE["nc"] = nc
    return nc


def _prep_core_inputs(tokens, Wq, bq, Wk, bk, Wv, bv, Wo, bo):
    """Host-side marshaling: slice per core, transpose tokens, cast bf16."""
    scale = 1.0 / np.sqrt(HEAD_DIM)
    per_batch_tokT = [
        np.ascontiguousarray(tokens[b].T).astype(BF16) for b in range(B)
    ]
    in_maps = []
    for core in range(N_CORES):
        b, g = divmod(core, GROUPS)
        cols = slice(g * DH, (g + 1) * DH)
        # weights as [128, 8*DH]: din-tile t at column block t
        wq_s = (Wq[:, cols] * scale).astype(BF16).reshape(8, 128, DH)
        wq_s = np.ascontiguousarray(wq_s.transpose(1, 0, 2)).reshape(128, 8 * DH)
        wk_s = Wk[:, cols].astype(BF16).reshape(8, 128, DH)
        wk_s = np.ascontiguousarray(wk_s.transpose(1, 0, 2)).reshape(128, 8 * DH)
        wv_s = Wv[:, cols].astype(BF16).reshape(8, 128, DH)
        wv_s = np.ascontiguousarray(wv_s.transpose(1, 0, 2)).reshape(128, 8 * DH)
        # Wo rows for this group, pair p at column block p
        wo_s = Wo[cols, :].astype(BF16).reshape(2, 128, D)
        wo_s = np.ascontiguousarray(wo_s.transpose(1, 0, 2)).reshape(128, 2 * D)
        # biases: column j = q-pair j (rows 0:64 even head, 64:128 odd),
        # column 2+j = k-pair j
        bqk_s = np.zeros((128, 4), np.float32)
        for j in range(2):
            bqk_s[0:64, j] = bq[g * DH + (2 * j) * 64:g * DH + (2 * j + 1) * 64] * scale
            bqk_s[64:128, j] = bq[g * DH + (2 * j + 1) * 64:g * DH + (2 * j + 2) * 64] * scale
            bqk_s[0:64, 2 + j] = bk[g * DH + (2 * j) * 64:g * DH + (2 * j + 1) * 64]
            bqk_s[64:128, 2 + j] = bk[g * DH + (2 * j + 1) * 64:g * DH + (2 * j + 2) * 64]
        in_maps.append({
            "tokT": per_batch_tokT[b],
            "wq": wq_s, "wk": wk_s, "wv": wv_s, "wo": wo_s,
            "bqk": bqk_s,
        })
    return in_maps


def kernel(tokens, Wq, bq, Wk, bk, Wv, bv, Wo, bo):
    tokens = np.asarray(tokens, np.float32)
    Wq = np.asarray(Wq, np.float32); bq = np.asarray(bq, np.float32)
    Wk = np.asarray(Wk, np.float32); bk = np.asarray(bk, np.float32)
    Wv = np.asarray(Wv, np.float32); bv = np.asarray(bv, np.float32)
    Wo = np.asarray(Wo, np.float32); bo = np.asarray(bo, np.float32)

    nc = _build()
    in_maps = _prep_core_inputs(tokens, Wq, bq, Wk, bk, Wv, bv, Wo, bo)
    res = bass_utils.run_bass_kernel_spmd(
        nc, in_maps, core_ids=list(range(N_CORES)))
    _CACHE["last_result"] = res

    bo_eff = (bv.astype(np.float64) @ Wo.astype(np.float64)
              + bo.astype(np.float64)).astype(np.float32)

    out = []
    for name in ("outT_neg", "outT_pos"):
        full = np.empty((B, S, D), np.float32)
        for b in range(B):
            acc = res.results[b * GROUPS][name].astype(np.float32)
            for g in range(1, GROUPS):
                acc += res.results[b * GROUPS + g][name].astype(np.float32)
            full[b] = acc.T
        full += bo_eff
        out.append(full)
    return tuple(out)


# revision 26
# speedup vs baseline: 1.0022x; 1.0022x over previous
"""Bidirectional attention (softmax(+logits) and softmax(-logits) branches)
on 8 Trainium2 NeuronCores.

Sharding: batch x head-group. Core c handles batch c//4 and heads
4*(c%4) .. 4*(c%4)+3. Each core computes its heads' Q/K/V projections,
both softmax branches, and a partial output projection (row-shard of Wo);
the host sums the 4 partials per batch and transposes.

All matmuls run in bf16 (fp32 matmul is 4x slower on the PE); PSUM
accumulation is fp32. The softmax uses unnormalized exp (logit range is
~N(0,1), no max-subtraction needed): P_pos = exp(+logits) via wide ACT
ops; P_neg = 1/P_pos via a single custom-DVE op (BITWISE_NOT seed + 2
inline Newton passes) run on the bf16 tiles directly -- this halves the
ACT engine's exp volume. Z comes free from a ones-column appended to V;
the 1/Z normalization is batched across both branches.

Pipeline structure (all HW-measured):
- ONE unified PSUM allocation: lgp (2x[128,1024]) + ftp (2x[128,1024])
  = 8 banks, live for the whole kernel.  Phases 1/3 borrow tiles from
  these rotations instead of opening their own pools (separate pools
  alias banks and serialize phase boundaries).  Matmul psum outputs
  must be BANK-aligned (mid-bank outputs corrupt on HW).
- Head-0 PRELUDE: the first head's logits/exp/recip are emitted before
  the V projection with its PV matmuls deferred, so the ACT exp stream
  starts as soon as Q/K are projected (~25us) instead of after the
  whole V projection (~50us).  The PV backlog drains 2-groups-per-
  iteration during head 1.
- PV matmuls trail their logits: pos by 1 group, neg by 2.  The serial
  chains logits -> exp(ACT) -> PV-pos (~1.1us) and -> recip(DVE) ->
  PV-neg (~2.2us) would otherwise stall the in-order PE queue every
  group.  PSUM accumulation order within a region is free; only the
  start (st==0) / stop (st==SK-1) flags matter.
- The ACT-computed neg-branch exps sit at g==3/4 (mid-head), away from
  the head-boundary congestion.
- Output projection (phase 3) alternates its borrowed psum pair between
  the lg and ft rotations (4 tiles in flight) and splits each evac into
  concurrent DVE+ACT halves, so it runs matmul-paced.
- full-width head-pair projections: one P_out=128 matmul covers both
  heads of a pair; Q^T/K^T kept in base-0 and base-64 copies so
  consecutive K=64 logit matmuls alternate PE row-groups.

Host-side prep folds the 1/sqrt(d) scale into Wq, and bv@Wo+bo into a
host-side bias (exact because softmax rows sum to 1).
"""

import os
import sys

for _p in ("/opt/trn_rl_repo",):
    if _p not in sys.path:
        sys.path.insert(0, _p)

import numpy as np
import ml_dtypes

import concourse.bass as bass
import concourse.tile as tile
from concourse import bacc, mybir
from concourse import bass_utils
from concourse.dve_ops import RECIPROCAL_APPROX_FAST, RECIP_APPROX_FAST_CONSTS

BF16 = ml_dtypes.bfloat16

B, S, D = 2, 2048, 1024
NUM_HEADS, HEAD_DIM = 16, 64
N_CORES = 8
GROUPS = 4                      # head groups (one per core within a batch)
H = NUM_HEADS // GROUPS         # heads per core = 4
DH = H * HEAD_DIM               # per-core head dims = 256
QCH = 512                       # q-chunk (matmul moving free dim)
NCH = S // QCH                  # 4 q-chunks
SK = S // 128                   # 16 s_k tiles
VSTRIDE = H * 128               # V' row stride: 4 heads x (64 v + 64 ones)

f32 = mybir.dt.float32
bf16 = mybir.dt.bfloat16
EXP = mybir.ActivationFunctionType.Exp
IDENT = mybir.ActivationFunctionType.Identity

# How many groups the PV matmuls trail their group's logits.
POS_DELAY = 1
NEG_DELAY = 2
# Max deferred-PV groups flushed per ring iteration (drains the head-0
# prelude backlog without monopolizing the PE).
FLUSH_CAP = 2

_CACHE = {}


def _emit(tc):
    nc = tc.nc
    tokT = nc.dram_tensor("tokT", [D, S], bf16, kind="ExternalInput").ap()
    wq = nc.dram_tensor("wq", [128, 8 * DH], bf16, kind="ExternalInput").ap()
    wk = nc.dram_tensor("wk", [128, 8 * DH], bf16, kind="ExternalInput").ap()
    wv = nc.dram_tensor("wv", [128, 8 * DH], bf16, kind="ExternalInput").ap()
    wo = nc.dram_tensor("wo", [128, 2 * D], bf16, kind="ExternalInput").ap()
    bqk = nc.dram_tensor("bqk", [128, 4], f32, kind="ExternalInput").ap()
    outs = [
        nc.dram_tensor("outT_neg", [D, S], bf16, kind="ExternalOutput").ap(),
        nc.dram_tensor("outT_pos", [D, S], bf16, kind="ExternalOutput").ap(),
    ]

    rc = RECIP_APPROX_FAST_CONSTS

    import contextlib

    with contextlib.ExitStack() as ctx:
        wp = ctx.enter_context(tc.tile_pool(name="wp", bufs=1))
        act = ctx.enter_context(tc.tile_pool(name="act", bufs=1))
        ppool = ctx.enter_context(tc.tile_pool(name="pp", bufs=2))
        fpool = ctx.enter_context(tc.tile_pool(name="fp", bufs=4))
        zpool = ctx.enter_context(tc.tile_pool(name="zp", bufs=2))
        opool = ctx.enter_context(tc.tile_pool(name="op", bufs=8))
        # the single 8-bank PSUM allocation for the whole kernel
        lgp = ctx.enter_context(tc.tile_pool(name="lgp", bufs=2, space="PSUM"))
        ftp = ctx.enter_context(tc.tile_pool(name="ftp", bufs=2, space="PSUM"))

        # ---- weight / bias / token loads -------------------------------
        # input loads in COMPUTE order: the K projection's t-chain chases
        # tok tiles arriving on sync, while the other weights ride the
        # gpsimd/scalar queues (HBM bandwidth is the real floor, so the
        # first-needed data must not share it with late-needed data)
        wq_sb = wp.tile([128, 8 * DH], bf16, tag="wq")
        wk_sb = wp.tile([128, 8 * DH], bf16, tag="wk")
        wv_sb = wp.tile([128, 8 * DH], bf16, tag="wv")
        wo_sb = wp.tile([128, 2 * D], bf16, tag="wo")
        bqk_sb = wp.tile([128, 4], f32, tag="bqk")
        # everything rides ONE queue in strict need-order so the
        # first-needed bytes get the full HBM bandwidth: wk+wq (the
        # concurrent K/Q t-chains), tok (their pacer), wv (V starts
        # after the Q evacs), wo (needed only at phase 3)
        nc.sync.dma_start(wk_sb[:], wk)
        nc.sync.dma_start(wq_sb[:], wq)
        nc.gpsimd.dma_start(bqk_sb[:], bqk)

        tok = []
        for t in range(8):
            tt = act.tile([128, S], bf16, tag=f"tok{t}")
            nc.sync.dma_start(tt[:], tokT[t * 128:(t + 1) * 128, :])
            tok.append(tt)
        nc.sync.dma_start(wv_sb[:], wv)
        nc.sync.dma_start(wo_sb[:], wo)

        # head-pair tiles: even head in partitions 0:64, odd in 64:128;
        # *_alt has the two halves swapped (so every head exists at both
        # partition bases -- lets logit matmuls alternate PE row groups)
        qt_pair = [act.tile([128, S], bf16, tag=f"qp{j}", name=f"qp{j}") for j in range(2)]
        kt_pair = [act.tile([128, S], bf16, tag=f"kp{j}", name=f"kp{j}") for j in range(2)]
        qt_alt = [act.tile([128, S], bf16, tag=f"qa{j}", name=f"qa{j}") for j in range(2)]
        kt_alt = [act.tile([128, S], bf16, tag=f"ka{j}", name=f"ka{j}") for j in range(2)]
        SKQ = SK // 4
        vp_q = [act.tile([128, SKQ * VSTRIDE], bf16, tag=f"vp{i}", name=f"vp{i}")
                for i in range(4)]

        def vap_for(st, h):
            q, r = divmod(st, SKQ)
            off = r * VSTRIDE + h * 128
            return vp_q[q][:, off:off + 128]

        # V' head block is [ones (64) | v (64)]: the 64 ones-columns make
        # the PV matmul write Z replicated across psum rows 0:64 -- a free
        # partition-broadcast of the softmax denominator (and a full
        # 128-col stationary enables FWL).  Z sits at base partition 0
        # because the custom-DVE reciprocal only works at base 0
        # (HW-verified); feat lands at rows 64:128.
        for q in range(4):
            ones_ap = vp_q[q].rearrange(
                "p (s h x) -> p (s h) x", s=SKQ, h=H)[:, :, 0:64]
            nc.gpsimd.memset(ones_ap, 1.0)

        def lgp_chains():
            tiles = [lgp.tile([128, GW * QCH], f32, tag="lg",
                              name=f"pjb{x}") for x in range(2)]
            return [tiles[x // 2][:, (x % 2) * QCH:(x % 2 + 1) * QCH]
                    for x in range(4)]

        def ftp_chains():
            tiles = [ftp.tile([128, GW * QCH], f32, tag="ft",
                              name="ft") for x in range(2)]
            return [tiles[x // 2][:, (x % 2) * QCH:(x % 2 + 1) * QCH]
                    for x in range(4)]

        # ---- phase 1: projections (borrowed ring psum tiles) -----------
        # (matmul psum outputs are capped at one bank = 512 f32 columns,
        # and must be bank-aligned)
        def qk_evac(j, pair, alt, ps, bcol0, on_dve):
            # evacs split across ACT (identity+bias) and DVE
            # (tensor_scalar add) so 8 of them don't serialize one engine
            for cc in range(4):
                dst = pair[j][:, cc * QCH:(cc + 1) * QCH]
                bias_ap = bqk_sb[:, bcol0 + j:bcol0 + j + 1]
                if on_dve:
                    nc.vector.tensor_scalar_add(dst, ps[cc], bias_ap)
                else:
                    nc.scalar.activation(dst, ps[cc], IDENT, bias=bias_ap)
            nc.sync.dma_start(alt[j][64:128, :], pair[j][0:64, :])
            nc.sync.dma_start(alt[j][0:64, :], pair[j][64:128, :])

        def qk_chain_t(ps, w_sb, base, t, first, last):
            for cc in range(4):
                nc.tensor.matmul(ps[cc], w_sb[:, base:base + 128],
                                 tok[t][:, cc * QCH:(cc + 1) * QCH],
                                 start=first, stop=last)

        def qk_proj(j):
            # full-width head-pair matmuls: one P_out=128 matmul per
            # (din-tile, chunk-pair) covers both heads (even head -> psum
            # rows 0:64, odd -> 64:128 -- weight cols are adjacent).
            # Pair 0 runs K and Q as CONCURRENT t-chains (K on the lg
            # psum pair, Q on the ft pair) so both finish with the token
            # DMA instead of Q serializing behind K's evac.  Pair 1 runs
            # mid-ring where the ft rotation is busy, so its Q chains
            # reuse the lg pair sequentially.
            base_j = j * 128
            ps_k = lgp_chains()
            ps_q = ftp_chains()
            for t in range(8):
                base = t * DH + base_j
                qk_chain_t(ps_k, wk_sb, base, t, t == 0, t == 7)
                qk_chain_t(ps_q, wq_sb, base, t, t == 0, t == 7)
            qk_evac(j, kt_pair, kt_alt, ps_k, 2, on_dve=False)
            qk_evac(j, qt_pair, qt_alt, ps_q, 0, on_dve=True)

        def emit_vproj():
            # V: 2 s-tile chains per sp-group, carved from one borrowed
            # ftp tile at BANK-ALIGNED offsets (0 and QCH*4B=2KB).
            # Matmul psum outputs at mid-bank offsets corrupt on HW, so
            # each [128,256] chain gets its own bank (half unused).
            for sp in range(SK // 2):
                ftv = ftp.tile([128, 2 * QCH], f32, tag="ft", name="ft")
                psv = [ftv[:, i * QCH:i * QCH + DH] for i in range(2)]
                for t in range(8):
                    for i in range(2):
                        st = sp * 2 + i
                        nc.tensor.matmul(
                            psv[i],
                            tok[t][:, st * 128:(st + 1) * 128],
                            wv_sb[:, t * DH:(t + 1) * DH],
                            start=(t == 0), stop=(t == 7),
                        )
                for i in range(2):
                    st = sp * 2 + i
                    q, r = divmod(st, SKQ)
                    dst = vp_q[q][:, r * VSTRIDE:(r + 1) * VSTRIDE]
                    dst3 = dst.rearrange("p (h x) -> p h x", h=H)[:, :, 64:128]
                    src3 = psv[i].rearrange("p (h x) -> p h x", h=H)
                    nc.vector.tensor_copy(dst3, src3)

        GW = 2           # s_k-tiles per exp group (FD = GW*QCH = 1024)
        NG = SK // GW    # 8 groups per (head, chunk)

        qk_proj(0)
        # pair-1 q/k follows in the DMA-bound prefix: its chains reuse
        # the lg/ft pairs right after pair-0's evacs, filling the PE
        # slack under the token DMA instead of adding ~14us to the ring
        qk_proj(1)
        # (the V projection is emitted at (c0,h1,g0), after head 0's
        # logits/exp prelude: the PE chews it while the ACT/DVE stream
        # is already running)

        # ---- phase 2: attention ----------------------------------------
        fsb_all = []
        # phase-3 output DMAs rotate over the three DMA-capable queues
        out_engines = [nc.sync, nc.gpsimd, nc.scalar]
        u_out = [0]

        # per-(c,h) feature psum, allocated lazily at the first deferred
        # PV flush so the ft rotation order matches actual use
        ft_map = {}

        def get_ft(c2, h2):
            if (c2, h2) not in ft_map:
                ft_map[(c2, h2)] = ftp.tile([128, 2 * QCH], f32, tag="ft",
                                            name="ft")
            return ft_map[(c2, h2)]

        def outproj_dp(c, dp):
            # one dout-pair iteration of the output projection: a psum
            # pair borrowed alternately from the lg / ft rotations (so 4
            # tiles pipeline), 2 dh-passes x 2 branches accumulation,
            # evac split into concurrent DVE (neg) + ACT (pos) halves,
            # output DMAs on sync/gpsimd
            pool, tg = (lgp, "lg") if dp % 2 == 0 else (ftp, "ft")
            ops = [pool.tile([128, GW * QCH], f32, tag=tg, name=f"ops{i}")
                   for i in range(2)]
            for p in range(2):
                for i in range(2):
                    dt = dp * 2 + i
                    lhs = wo_sb[:, p * D + dt * 128:p * D + (dt + 1) * 128]
                    for br in range(2):
                        nc.tensor.matmul(
                            ops[i][:, br * QCH:(br + 1) * QCH],
                            lhs,
                            fsb_all[c][p][:, br * QCH:(br + 1) * QCH],
                            start=(p == 0), stop=(p == 1),
                        )
            for i in range(2):
                dt = dp * 2 + i
                osb = opool.tile([128, 2 * QCH], bf16, tag="os", name="osb")
                nc.vector.tensor_copy(osb[:, 0:QCH], ops[i][:, 0:QCH])
                nc.scalar.copy(osb[:, QCH:2 * QCH], ops[i][:, QCH:2 * QCH])
                for br in range(2):
                    eng = out_engines[u_out[0] % len(out_engines)]
                    u_out[0] += 1
                    eng.dma_start(
                        outs[br][dt * 128:(dt + 1) * 128,
                                 c * QCH:(c + 1) * QCH],
                        osb[:, br * QCH:(br + 1) * QCH],
                    )

        # deferred normalization: the recip -> zhi-DMA -> mul chain of
        # head h is emitted split across head h+1's groups 3/5 (after
        # head h's delayed PV matmuls have all been flushed), so the
        # sync-DMA round trip overlaps queued recip work instead of
        # stalling the strict-FIFO DVE queue at each head boundary
        def norm_start(stt):
            c2, h2, jp, parp, fsbp = stt
            ft2p = ft_map[(c2, h2)]
            zb = zpool.tile([64, 2 * QCH], f32, tag="zb", name="zb")
            zhi = zpool.tile([128, 2 * QCH], f32, tag="zhi", name="zhi")
            nc.vector.reciprocal_approx_fast(zb[:], ft2p[0:64, :])
            nc.sync.dma_start(zhi[64:128, :], zb[:])
            return (ft2p, parp, fsbp, zhi)

        def norm_finish(stt):
            ft2p, parp, fsbp, zhi = stt
            if parp == 1:
                # odd head: feat rows 64:128 align with the fsb
                # destination -- write it directly
                nc.vector.tensor_mul(fsbp[64:128, :],
                                     ft2p[64:128, :], zhi[64:128, :])
            else:
                tmp = zpool.tile([128, 2 * QCH], bf16, tag="tmp",
                                 name="tmp")
                nc.vector.tensor_mul(tmp[64:128, :], ft2p[64:128, :],
                                     zhi[64:128, :])
                nc.sync.dma_start(fsbp[0:64, :], tmp[64:128, :])

        def emit_logits(c2, h2, g2):
            # K=64 logit matmul pair on alternating PE row groups
            j2, par2 = h2 // 2, h2 % 2
            k_lo = kt_pair[j2] if par2 == 0 else kt_alt[j2]
            k_hi = kt_alt[j2] if par2 == 0 else kt_pair[j2]
            q_lo = qt_pair[j2] if par2 == 0 else qt_alt[j2]
            q_hi = qt_alt[j2] if par2 == 0 else qt_pair[j2]
            lg = lgp.tile([128, GW * QCH], f32, tag="lg")
            for t2 in range(GW):
                st = g2 * GW + t2
                if st % 2 == 0:
                    nc.tensor.matmul(
                        lg[:, t2 * QCH:(t2 + 1) * QCH],
                        k_lo[0:64, st * 128:(st + 1) * 128],
                        q_lo[0:64, c2 * QCH:(c2 + 1) * QCH],
                        start=True, stop=True)
                else:
                    nc.tensor.matmul(
                        lg[:, t2 * QCH:(t2 + 1) * QCH],
                        k_hi[64:128, st * 128:(st + 1) * 128],
                        q_hi[64:128, c2 * QCH:(c2 + 1) * QCH],
                        start=True, stop=True)
            return lg

        def emit_pv(item, br):
            pw, c2, h2, g2 = item
            ft2_ = get_ft(c2, h2)
            for t2 in range(GW):
                st = g2 * GW + t2
                nc.tensor.matmul(
                    ft2_[:, br * QCH:(br + 1) * QCH],
                    vap_for(st, h2),
                    pw[:, t2 * QCH:(t2 + 1) * QCH],
                    start=(st == 0),
                    stop=(st == SK - 1),
                )

        # flattened group sequence.  Logits are software-pipelined ONE
        # group ahead; PV-pos trails POS_DELAY and PV-neg NEG_DELAY
        # groups so the ACT/DVE chains are covered by other PE work.
        # Head (c0,h0) is a PRELUDE: its PV is fully deferred past the
        # V projection.
        groups = [(c, h, g) for c in range(NCH) for h in range(H)
                  for g in range(NG)]
        lg_next = None
        pend = pend2 = None
        pend_pos = []
        pend_neg = []

        for i, (c, h, g) in enumerate(groups):
            j, par = h // 2, h % 2
            if g == 0 and h == 0:
                fsb = [fpool.tile([128, 2 * QCH], bf16, tag="fsb",
                                  bufs=8, name=f"fsb{j2}")
                       for j2 in range(2)]
                fsb_all.append(fsb)
                fsb_c = fsb
            if c == 0 and h == 1 and g == 0:
                # the deferred V projection (8 ft-tile sp-groups),
                # emitted under head 0's already-running exp/recip stream
                emit_vproj()
            if i == 0:
                lg_next = emit_logits(c, h, g)
            lg = lg_next
            if i + 1 < len(groups):
                lg_next = emit_logits(*groups[i + 1])
            pw_pos = ppool.tile([128, GW * QCH], bf16, tag="pwp",
                                name="pwp", bufs=11)
            pw_neg = ppool.tile([128, GW * QCH], bf16, tag="pwn",
                                name="pwn", bufs=12)
            nc.scalar.activation(pw_pos[:], lg[:], EXP)
            if g == 3 or (g == 4 and ((h + c) & 1)):
                # mid-head ACT-exp groups balance ACT vs DVE without
                # piling a second exp onto the busy head-boundary groups
                nc.scalar.activation(pw_neg[:], lg[:], EXP, scale=-1.0)
            else:
                # P_neg = 1/P_pos on DVE (bf16 in/out; the DVE read
                # path converts bf16->fp32 bit-exactly so the fp32
                # BITWISE_NOT seed applies unchanged)
                nc.vector._custom_dve(
                    RECIPROCAL_APPROX_FAST,
                    out=pw_neg[:], in0=pw_pos[:],
                    s0=rc["s0"], s1=rc["s1"], imm2=rc["imm2"],
                )
            if not (c == 0 and h == 0):
                # flush deferred PV: neg first (its recip is oldest)
                for _ in range(FLUSH_CAP):
                    if len(pend_neg) > NEG_DELAY:
                        emit_pv(pend_neg.pop(0), 0)
                for _ in range(FLUSH_CAP):
                    if len(pend_pos) > POS_DELAY:
                        emit_pv(pend_pos.pop(0), 1)
            pend_pos.append((pw_pos, c, h, g))
            pend_neg.append((pw_neg, c, h, g))
            # previous head's deferred normalization: feat / Z.  Z sits
            # pre-broadcast in its psum rows 0:64 (the ones-columns),
            # feat in rows 64:128, both branches in one [64, 2*QCH] op.
            # recip at base 0 (the custom-DVE op requires it), one SBUF
            # DMA shifts 1/Z up to rows 64:128, multiply straight out of
            # PSUM.
            if g == 3 and pend is not None:
                pend2, pend = norm_start(pend), None
            elif g == 5 and pend2 is not None:
                norm_finish(pend2)
                pend2 = None
            if g == NG - 1:
                pend = (c, h, j, par, fsb_c[j])

        while pend_pos or pend_neg:
            if pend_neg:
                emit_pv(pend_neg.pop(0), 0)
            if pend_pos:
                emit_pv(pend_pos.pop(0), 1)
        norm_finish(norm_start(pend))

        # ---- phase 3: output projection (borrowed lg/ft psum) ----------
        for c in range(NCH):
            for dp in range(4):
                outproj_dp(c, dp)


def _build():
    if "nc" in _CACHE:
        return _CACHE["nc"]
    nc = bacc.Bacc("TRN2", target_bir_lowering=False, debug=False,
                   num_devices=N_CORES)
    with tile.TileContext(nc) as tc:
        _emit(tc)
    nc.compile()
    _CACHE["nc"] = nc
    return nc


def _prep_core_inputs(tokens, Wq, bq, Wk, bk, Wv, bv, Wo, bo):
    """Host-side marshaling: slice per core, transpose tokens, cast bf16."""
    scale = 1.0 / np.sqrt(HEAD_DIM)
    per_batch_tokT = [
        np.ascontiguousarray(tokens[b].T).astype(BF16) for b in range(B)
    ]
    in_maps = []
    for core in range(N_CORES):
        b, g = divmod(core, GROUPS)
        cols = slice(g * DH, (g + 1) * DH)
        # weights as [128, 8*DH]: din-tile t at column block t
        wq_s = (Wq[:, cols] * scale).astype(BF16).reshape(8, 128, DH)
        wq_s = np.ascontiguousarray(wq_s.transpose(1, 0, 2)).reshape(128, 8 * DH)
        wk_s = Wk[:, cols].astype(BF16).reshape(8, 128, DH)
        wk_s = np.ascontiguousarray(wk_s.transpose(1, 0, 2)).reshape(128, 8 * DH)
        wv_s = Wv[:, cols].astype(BF16).reshape(8, 128, DH)
        wv_s = np.ascontiguousarray(wv_s.transpose(1, 0, 2)).reshape(128, 8 * DH)
        # Wo rows for this group, pair p at column block p
        wo_s = Wo[cols, :].astype(BF16).reshape(2, 128, D)
        wo_s = np.ascontiguousarray(wo_s.transpose(1, 0, 2)).reshape(128, 2 * D)
        # biases: column j = q-pair j (rows 0:64 even head, 64:128 odd),
        # column 2+j = k-pair j
        bqk_s = np.zeros((128, 4), np.float32)
        for j in range(2):
            bqk_s[0:64, j] = bq[g * DH + (2 * j) * 64:g * DH + (2 * j + 1) * 64] * scale
            bqk_s[64:128, j] = bq[g * DH + (2 * j + 1) * 64:g * DH + (2 * j + 2) * 64] * scale
            bqk_s[0:64, 2 + j] = bk[g * DH + (2 * j) * 64:g * DH + (2 * j + 1) * 64]
            bqk_s[64:128, 2 + j] = bk[g * DH + (2 * j + 1) * 64:g * DH + (2 * j + 2) * 64]
        in_maps.append({
            "tokT": per_batch_tokT[b],
            "wq": wq_s, "wk": wk_s, "wv": wv_s, "wo": wo_s,
            "bqk": bqk_s,
        })
    return in_maps


def kernel(tokens, Wq, bq, Wk, bk, Wv, bv, Wo, bo):
    tokens = np.asarray(tokens, np.float32)
    Wq = np.asarray(Wq, np.float32); bq = np.asarray(bq, np.float32)
    Wk = np.asarray(Wk, np.float32); bk = np.asarray(bk, np.float32)
    Wv = np.asarray(Wv, np.float32); bv = np.asarray(bv, np.float32)
    Wo = np.asarray(Wo, np.float32); bo = np.asarray(bo, np.float32)

    nc = _build()
    in_maps = _prep_core_inputs(tokens, Wq, bq, Wk, bk, Wv, bv, Wo, bo)
    res = bass_utils.run_bass_kernel_spmd(
        nc, in_maps, core_ids=list(range(N_CORES)))
    _CACHE["last_result"] = res

    bo_eff = (bv.astype(np.float64) @ Wo.astype(np.float64)
              + bo.astype(np.float64)).astype(np.float32)

    out = []
    for name in ("outT_neg", "outT_pos"):
        full = np.empty((B, S, D), np.float32)
        for b in range(B):
            acc = res.results[b * GROUPS][name].astype(np.float32)
            for g in range(1, GROUPS):
                acc += res.results[b * GROUPS + g][name].astype(np.float32)
            full[b] = acc.T
        full += bo_eff
        out.append(full)
    return tuple(out)


# revision 27
# speedup vs baseline: 1.0066x; 1.0044x over previous
"""Bidirectional attention (softmax(+logits) and softmax(-logits) branches)
on 8 Trainium2 NeuronCores.

Sharding: batch x head-group. Core c handles batch c//4 and heads
4*(c%4) .. 4*(c%4)+3. Each core computes its heads' Q/K/V projections,
both softmax branches, and a partial output projection (row-shard of Wo);
the host sums the 4 partials per batch and transposes.

All matmuls run in bf16 (fp32 matmul is 4x slower on the PE); PSUM
accumulation is fp32. The softmax uses unnormalized exp (logit range is
~N(0,1), no max-subtraction needed): P_pos = exp(+logits) via wide ACT
ops; P_neg = 1/P_pos via a single custom-DVE op (BITWISE_NOT seed + 2
inline Newton passes) run on the bf16 tiles directly -- this halves the
ACT engine's exp volume. Z comes free from a ones-column appended to V;
the 1/Z normalization is batched across both branches.

Pipeline structure (all HW-measured):
- ONE unified PSUM allocation: lgp (2x[128,1024]) + ftp (2x[128,1024])
  = 8 banks, live for the whole kernel.  Phases 1/3 borrow tiles from
  these rotations instead of opening their own pools (separate pools
  alias banks and serialize phase boundaries).  Matmul psum outputs
  must be BANK-aligned (mid-bank outputs corrupt on HW).
- Head-0 PRELUDE: the first head's logits/exp/recip are emitted before
  the V projection with its PV matmuls deferred, so the ACT exp stream
  starts as soon as Q/K are projected (~25us) instead of after the
  whole V projection (~50us).  The PV backlog drains 2-groups-per-
  iteration during head 1.
- PV matmuls trail their logits: pos by 1 group, neg by 2.  The serial
  chains logits -> exp(ACT) -> PV-pos (~1.1us) and -> recip(DVE) ->
  PV-neg (~2.2us) would otherwise stall the in-order PE queue every
  group.  PSUM accumulation order within a region is free; only the
  start (st==0) / stop (st==SK-1) flags matter.
- The ACT-computed neg-branch exps sit at g==3/4 (mid-head), away from
  the head-boundary congestion.
- Output projection (phase 3) alternates its borrowed psum pair between
  the lg and ft rotations (4 tiles in flight) and splits each evac into
  concurrent DVE+ACT halves, so it runs matmul-paced.
- full-width head-pair projections: one P_out=128 matmul covers both
  heads of a pair; Q^T/K^T kept in base-0 and base-64 copies so
  consecutive K=64 logit matmuls alternate PE row-groups.

Host-side prep folds the 1/sqrt(d) scale into Wq, and bv@Wo+bo into a
host-side bias (exact because softmax rows sum to 1).
"""

import os
import sys

for _p in ("/opt/trn_rl_repo",):
    if _p not in sys.path:
        sys.path.insert(0, _p)

import numpy as np
import ml_dtypes

import concourse.bass as bass
import concourse.tile as tile
from concourse import bacc, mybir
from concourse import bass_utils
from concourse.dve_ops import RECIPROCAL_APPROX_FAST, RECIP_APPROX_FAST_CONSTS

BF16 = ml_dtypes.bfloat16

B, S, D = 2, 2048, 1024
NUM_HEADS, HEAD_DIM = 16, 64
N_CORES = 8
GROUPS = 4                      # head groups (one per core within a batch)
H = NUM_HEADS // GROUPS         # heads per core = 4
DH = H * HEAD_DIM               # per-core head dims = 256
QCH = 512                       # q-chunk (matmul moving free dim)
NCH = S // QCH                  # 4 q-chunks
SK = S // 128                   # 16 s_k tiles
VSTRIDE = H * 128               # V' row stride: 4 heads x (64 v + 64 ones)

f32 = mybir.dt.float32
bf16 = mybir.dt.bfloat16
EXP = mybir.ActivationFunctionType.Exp
IDENT = mybir.ActivationFunctionType.Identity

# How many groups the PV matmuls trail their group's logits.
POS_DELAY = 2
NEG_DELAY = 3
# Max deferred-PV groups flushed per ring iteration (drains the head-0
# prelude backlog without monopolizing the PE).
FLUSH_CAP = 3

_CACHE = {}


def _emit(tc):
    nc = tc.nc
    tokT = nc.dram_tensor("tokT", [D, S], bf16, kind="ExternalInput").ap()
    wq = nc.dram_tensor("wq", [128, 8 * DH], bf16, kind="ExternalInput").ap()
    wk = nc.dram_tensor("wk", [128, 8 * DH], bf16, kind="ExternalInput").ap()
    wv = nc.dram_tensor("wv", [128, 8 * DH], bf16, kind="ExternalInput").ap()
    wo = nc.dram_tensor("wo", [128, 2 * D], bf16, kind="ExternalInput").ap()
    bqk = nc.dram_tensor("bqk", [128, 4], f32, kind="ExternalInput").ap()
    outs = [
        nc.dram_tensor("outT_neg", [D, S], bf16, kind="ExternalOutput").ap(),
        nc.dram_tensor("outT_pos", [D, S], bf16, kind="ExternalOutput").ap(),
    ]

    rc = RECIP_APPROX_FAST_CONSTS

    import contextlib

    with contextlib.ExitStack() as ctx:
        wp = ctx.enter_context(tc.tile_pool(name="wp", bufs=1))
        act = ctx.enter_context(tc.tile_pool(name="act", bufs=1))
        ppool = ctx.enter_context(tc.tile_pool(name="pp", bufs=2))
        fpool = ctx.enter_context(tc.tile_pool(name="fp", bufs=4))
        zpool = ctx.enter_context(tc.tile_pool(name="zp", bufs=2))
        opool = ctx.enter_context(tc.tile_pool(name="op", bufs=8))
        # the single 8-bank PSUM allocation for the whole kernel
        lgp = ctx.enter_context(tc.tile_pool(name="lgp", bufs=2, space="PSUM"))
        ftp = ctx.enter_context(tc.tile_pool(name="ftp", bufs=2, space="PSUM"))

        # ---- weight / bias / token loads -------------------------------
        # input loads in COMPUTE order: the K projection's t-chain chases
        # tok tiles arriving on sync, while the other weights ride the
        # gpsimd/scalar queues (HBM bandwidth is the real floor, so the
        # first-needed data must not share it with late-needed data)
        wq_sb = wp.tile([128, 8 * DH], bf16, tag="wq")
        wk_sb = wp.tile([128, 8 * DH], bf16, tag="wk")
        wv_sb = wp.tile([128, 8 * DH], bf16, tag="wv")
        wo_sb = wp.tile([128, 2 * D], bf16, tag="wo")
        bqk_sb = wp.tile([128, 4], f32, tag="bqk")
        # everything rides ONE queue in strict need-order so the
        # first-needed bytes get the full HBM bandwidth: wk+wq (the
        # concurrent K/Q t-chains), tok (their pacer), wv (V starts
        # after the Q evacs), wo (needed only at phase 3)
        nc.sync.dma_start(wk_sb[:], wk)
        nc.sync.dma_start(wq_sb[:], wq)
        nc.gpsimd.dma_start(bqk_sb[:], bqk)

        tok = []
        for t in range(8):
            tt = act.tile([128, S], bf16, tag=f"tok{t}")
            nc.sync.dma_start(tt[:], tokT[t * 128:(t + 1) * 128, :])
            tok.append(tt)
        nc.sync.dma_start(wv_sb[:], wv)
        nc.sync.dma_start(wo_sb[:], wo)

        # head-pair tiles: even head in partitions 0:64, odd in 64:128;
        # *_alt has the two halves swapped (so every head exists at both
        # partition bases -- lets logit matmuls alternate PE row groups)
        qt_pair = [act.tile([128, S], bf16, tag=f"qp{j}", name=f"qp{j}") for j in range(2)]
        kt_pair = [act.tile([128, S], bf16, tag=f"kp{j}", name=f"kp{j}") for j in range(2)]
        qt_alt = [act.tile([128, S], bf16, tag=f"qa{j}", name=f"qa{j}") for j in range(2)]
        kt_alt = [act.tile([128, S], bf16, tag=f"ka{j}", name=f"ka{j}") for j in range(2)]
        SKQ = SK // 4
        vp_q = [act.tile([128, SKQ * VSTRIDE], bf16, tag=f"vp{i}", name=f"vp{i}")
                for i in range(4)]

        def vap_for(st, h):
            q, r = divmod(st, SKQ)
            off = r * VSTRIDE + h * 128
            return vp_q[q][:, off:off + 128]

        # V' head block is [ones (64) | v (64)]: the 64 ones-columns make
        # the PV matmul write Z replicated across psum rows 0:64 -- a free
        # partition-broadcast of the softmax denominator (and a full
        # 128-col stationary enables FWL).  Z sits at base partition 0
        # because the custom-DVE reciprocal only works at base 0
        # (HW-verified); feat lands at rows 64:128.
        for q in range(4):
            ones_ap = vp_q[q].rearrange(
                "p (s h x) -> p (s h) x", s=SKQ, h=H)[:, :, 0:64]
            nc.gpsimd.memset(ones_ap, 1.0)

        def lgp_chains():
            tiles = [lgp.tile([128, GW * QCH], f32, tag="lg",
                              name=f"pjb{x}") for x in range(2)]
            return [tiles[x // 2][:, (x % 2) * QCH:(x % 2 + 1) * QCH]
                    for x in range(4)]

        def ftp_chains():
            tiles = [ftp.tile([128, GW * QCH], f32, tag="ft",
                              name="ft") for x in range(2)]
            return [tiles[x // 2][:, (x % 2) * QCH:(x % 2 + 1) * QCH]
                    for x in range(4)]

        # ---- phase 1: projections (borrowed ring psum tiles) -----------
        # (matmul psum outputs are capped at one bank = 512 f32 columns,
        # and must be bank-aligned)
        def qk_evac(j, pair, alt, ps, bcol0, on_dve):
            # evacs split across ACT (identity+bias) and DVE
            # (tensor_scalar add) so 8 of them don't serialize one engine
            for cc in range(4):
                dst = pair[j][:, cc * QCH:(cc + 1) * QCH]
                bias_ap = bqk_sb[:, bcol0 + j:bcol0 + j + 1]
                if on_dve:
                    nc.vector.tensor_scalar_add(dst, ps[cc], bias_ap)
                else:
                    nc.scalar.activation(dst, ps[cc], IDENT, bias=bias_ap)
            nc.sync.dma_start(alt[j][64:128, :], pair[j][0:64, :])
            nc.sync.dma_start(alt[j][0:64, :], pair[j][64:128, :])

        def qk_chain_t(ps, w_sb, base, t, first, last):
            for cc in range(4):
                nc.tensor.matmul(ps[cc], w_sb[:, base:base + 128],
                                 tok[t][:, cc * QCH:(cc + 1) * QCH],
                                 start=first, stop=last)

        def qk_proj(j):
            # full-width head-pair matmuls: one P_out=128 matmul per
            # (din-tile, chunk-pair) covers both heads (even head -> psum
            # rows 0:64, odd -> 64:128 -- weight cols are adjacent).
            # Pair 0 runs K and Q as CONCURRENT t-chains (K on the lg
            # psum pair, Q on the ft pair) so both finish with the token
            # DMA instead of Q serializing behind K's evac.  Pair 1 runs
            # mid-ring where the ft rotation is busy, so its Q chains
            # reuse the lg pair sequentially.
            base_j = j * 128
            ps_k = lgp_chains()
            ps_q = ftp_chains()
            for t in range(8):
                base = t * DH + base_j
                qk_chain_t(ps_k, wk_sb, base, t, t == 0, t == 7)
                qk_chain_t(ps_q, wq_sb, base, t, t == 0, t == 7)
            qk_evac(j, kt_pair, kt_alt, ps_k, 2, on_dve=False)
            qk_evac(j, qt_pair, qt_alt, ps_q, 0, on_dve=True)

        def emit_vproj():
            # V: 2 s-tile chains per sp-group, carved from one borrowed
            # ftp tile at BANK-ALIGNED offsets (0 and QCH*4B=2KB).
            # Matmul psum outputs at mid-bank offsets corrupt on HW, so
            # each [128,256] chain gets its own bank (half unused).
            for sp in range(SK // 2):
                ftv = ftp.tile([128, 2 * QCH], f32, tag="ft", name="ft")
                psv = [ftv[:, i * QCH:i * QCH + DH] for i in range(2)]
                for t in range(8):
                    for i in range(2):
                        st = sp * 2 + i
                        nc.tensor.matmul(
                            psv[i],
                            tok[t][:, st * 128:(st + 1) * 128],
                            wv_sb[:, t * DH:(t + 1) * DH],
                            start=(t == 0), stop=(t == 7),
                        )
                for i in range(2):
                    st = sp * 2 + i
                    q, r = divmod(st, SKQ)
                    dst = vp_q[q][:, r * VSTRIDE:(r + 1) * VSTRIDE]
                    dst3 = dst.rearrange("p (h x) -> p h x", h=H)[:, :, 64:128]
                    src3 = psv[i].rearrange("p (h x) -> p h x", h=H)
                    nc.vector.tensor_copy(dst3, src3)

        GW = 2           # s_k-tiles per exp group (FD = GW*QCH = 1024)
        NG = SK // GW    # 8 groups per (head, chunk)

        qk_proj(0)
        # pair-1 q/k follows in the DMA-bound prefix: its chains reuse
        # the lg/ft pairs right after pair-0's evacs, filling the PE
        # slack under the token DMA instead of adding ~14us to the ring
        qk_proj(1)
        # (the V projection is emitted at (c0,h1,g0), after head 0's
        # logits/exp prelude: the PE chews it while the ACT/DVE stream
        # is already running)

        # ---- phase 2: attention ----------------------------------------
        fsb_all = []
        # phase-3 output DMAs rotate over the three DMA-capable queues
        out_engines = [nc.sync, nc.gpsimd, nc.scalar]
        u_out = [0]

        # per-(c,h) feature psum, allocated lazily at the first deferred
        # PV flush so the ft rotation order matches actual use
        ft_map = {}

        def get_ft(c2, h2):
            if (c2, h2) not in ft_map:
                ft_map[(c2, h2)] = ftp.tile([128, 2 * QCH], f32, tag="ft",
                                            name="ft")
            return ft_map[(c2, h2)]

        def outproj_dp(c, dp):
            # one dout-pair iteration of the output projection: a psum
            # pair borrowed alternately from the lg / ft rotations (so 4
            # tiles pipeline), 2 dh-passes x 2 branches accumulation,
            # evac split into concurrent DVE (neg) + ACT (pos) halves,
            # output DMAs on sync/gpsimd
            pool, tg = (lgp, "lg") if dp % 2 == 0 else (ftp, "ft")
            ops = [pool.tile([128, GW * QCH], f32, tag=tg, name=f"ops{i}")
                   for i in range(2)]
            for p in range(2):
                for i in range(2):
                    dt = dp * 2 + i
                    lhs = wo_sb[:, p * D + dt * 128:p * D + (dt + 1) * 128]
                    for br in range(2):
                        nc.tensor.matmul(
                            ops[i][:, br * QCH:(br + 1) * QCH],
                            lhs,
                            fsb_all[c][p][:, br * QCH:(br + 1) * QCH],
                            start=(p == 0), stop=(p == 1),
                        )
            for i in range(2):
                dt = dp * 2 + i
                osb = opool.tile([128, 2 * QCH], bf16, tag="os", name="osb")
                nc.vector.tensor_copy(osb[:, 0:QCH], ops[i][:, 0:QCH])
                nc.scalar.copy(osb[:, QCH:2 * QCH], ops[i][:, QCH:2 * QCH])
                for br in range(2):
                    eng = out_engines[u_out[0] % len(out_engines)]
                    u_out[0] += 1
                    eng.dma_start(
                        outs[br][dt * 128:(dt + 1) * 128,
                                 c * QCH:(c + 1) * QCH],
                        osb[:, br * QCH:(br + 1) * QCH],
                    )

        # deferred normalization: the recip -> zhi-DMA -> mul chain of
        # head h is emitted split across head h+1's groups 3/5 (after
        # head h's delayed PV matmuls have all been flushed), so the
        # sync-DMA round trip overlaps queued recip work instead of
        # stalling the strict-FIFO DVE queue at each head boundary
        def norm_start(stt):
            c2, h2, jp, parp, fsbp = stt
            ft2p = ft_map[(c2, h2)]
            zb = zpool.tile([64, 2 * QCH], f32, tag="zb", name="zb")
            zhi = zpool.tile([128, 2 * QCH], f32, tag="zhi", name="zhi")
            nc.vector.reciprocal_approx_fast(zb[:], ft2p[0:64, :])
            nc.sync.dma_start(zhi[64:128, :], zb[:])
            return (ft2p, parp, fsbp, zhi)

        def norm_finish(stt):
            ft2p, parp, fsbp, zhi = stt
            if parp == 1:
                # odd head: feat rows 64:128 align with the fsb
                # destination -- write it directly
                nc.vector.tensor_mul(fsbp[64:128, :],
                                     ft2p[64:128, :], zhi[64:128, :])
            else:
                tmp = zpool.tile([128, 2 * QCH], bf16, tag="tmp",
                                 name="tmp")
                nc.vector.tensor_mul(tmp[64:128, :], ft2p[64:128, :],
                                     zhi[64:128, :])
                nc.sync.dma_start(fsbp[0:64, :], tmp[64:128, :])

        def emit_logits(c2, h2, g2):
            # K=64 logit matmul pair on alternating PE row groups
            j2, par2 = h2 // 2, h2 % 2
            k_lo = kt_pair[j2] if par2 == 0 else kt_alt[j2]
            k_hi = kt_alt[j2] if par2 == 0 else kt_pair[j2]
            q_lo = qt_pair[j2] if par2 == 0 else qt_alt[j2]
            q_hi = qt_alt[j2] if par2 == 0 else qt_pair[j2]
            lg = lgp.tile([128, GW * QCH], f32, tag="lg")
            for t2 in range(GW):
                st = g2 * GW + t2
                if st % 2 == 0:
                    nc.tensor.matmul(
                        lg[:, t2 * QCH:(t2 + 1) * QCH],
                        k_lo[0:64, st * 128:(st + 1) * 128],
                        q_lo[0:64, c2 * QCH:(c2 + 1) * QCH],
                        start=True, stop=True)
                else:
                    nc.tensor.matmul(
                        lg[:, t2 * QCH:(t2 + 1) * QCH],
                        k_hi[64:128, st * 128:(st + 1) * 128],
                        q_hi[64:128, c2 * QCH:(c2 + 1) * QCH],
                        start=True, stop=True)
            return lg

        def emit_pv(item, br):
            pw, c2, h2, g2 = item
            ft2_ = get_ft(c2, h2)
            for t2 in range(GW):
                st = g2 * GW + t2
                nc.tensor.matmul(
                    ft2_[:, br * QCH:(br + 1) * QCH],
                    vap_for(st, h2),
                    pw[:, t2 * QCH:(t2 + 1) * QCH],
                    start=(st == 0),
                    stop=(st == SK - 1),
                )

        # flattened group sequence.  Logits are software-pipelined ONE
        # group ahead; PV-pos trails POS_DELAY and PV-neg NEG_DELAY
        # groups so the ACT/DVE chains are covered by other PE work.
        # Head (c0,h0) is a PRELUDE: its PV is fully deferred past the
        # V projection.
        groups = [(c, h, g) for c in range(NCH) for h in range(H)
                  for g in range(NG)]
        lg_next = None
        pend = pend2 = None
        pend_pos = []
        pend_neg = []

        for i, (c, h, g) in enumerate(groups):
            j, par = h // 2, h % 2
            if g == 0 and h == 0:
                fsb = [fpool.tile([128, 2 * QCH], bf16, tag="fsb",
                                  bufs=8, name=f"fsb{j2}")
                       for j2 in range(2)]
                fsb_all.append(fsb)
                fsb_c = fsb
            if c == 0 and h == 1 and g == 0:
                # the deferred V projection (8 ft-tile sp-groups),
                # emitted under head 0's already-running exp/recip stream
                emit_vproj()
            if i == 0:
                lg_next = emit_logits(c, h, g)
            lg = lg_next
            if i + 1 < len(groups):
                lg_next = emit_logits(*groups[i + 1])
            pw_pos = ppool.tile([128, GW * QCH], bf16, tag="pwp",
                                name="pwp", bufs=11)
            pw_neg = ppool.tile([128, GW * QCH], bf16, tag="pwn",
                                name="pwn", bufs=12)
            nc.scalar.activation(pw_pos[:], lg[:], EXP)
            if g == 3 or (g == 4 and ((h + c) & 1)):
                # mid-head ACT-exp groups balance ACT vs DVE without
                # piling a second exp onto the busy head-boundary groups
                nc.scalar.activation(pw_neg[:], lg[:], EXP, scale=-1.0)
            else:
                # P_neg = 1/P_pos on DVE (bf16 in/out; the DVE read
                # path converts bf16->fp32 bit-exactly so the fp32
                # BITWISE_NOT seed applies unchanged)
                nc.vector._custom_dve(
                    RECIPROCAL_APPROX_FAST,
                    out=pw_neg[:], in0=pw_pos[:],
                    s0=rc["s0"], s1=rc["s1"], imm2=rc["imm2"],
                )
            if not (c == 0 and h == 0):
                # flush deferred PV: neg first (its recip is oldest)
                for _ in range(FLUSH_CAP):
                    if len(pend_neg) > NEG_DELAY:
                        emit_pv(pend_neg.pop(0), 0)
                for _ in range(FLUSH_CAP):
                    if len(pend_pos) > POS_DELAY:
                        emit_pv(pend_pos.pop(0), 1)
            pend_pos.append((pw_pos, c, h, g))
            pend_neg.append((pw_neg, c, h, g))
            # previous head's deferred normalization: feat / Z.  Z sits
            # pre-broadcast in its psum rows 0:64 (the ones-columns),
            # feat in rows 64:128, both branches in one [64, 2*QCH] op.
            # recip at base 0 (the custom-DVE op requires it), one SBUF
            # DMA shifts 1/Z up to rows 64:128, multiply straight out of
            # PSUM.
            if g == 3 and pend is not None:
                pend2, pend = norm_start(pend), None
            elif g == 5 and pend2 is not None:
                norm_finish(pend2)
                pend2 = None
            if g == NG - 1:
                pend = (c, h, j, par, fsb_c[j])

        while pend_pos or pend_neg:
            if pend_neg:
                emit_pv(pend_neg.pop(0), 0)
            if pend_pos:
                emit_pv(pend_pos.pop(0), 1)
        norm_finish(norm_start(pend))

        # ---- phase 3: output projection (borrowed lg/ft psum) ----------
        for c in range(NCH):
            for dp in range(4):
                outproj_dp(c, dp)


def _build():
    if "nc" in _CACHE:
        return _CACHE["nc"]
    nc = bacc.Bacc("TRN2", target_bir_lowering=False, debug=False,
                   num_devices=N_CORES)
    with tile.TileContext(nc) as tc:
        _emit(tc)
    nc.compile()
    _CACHE["nc"] = nc
    return nc


def _prep_core_inputs(tokens, Wq, bq, Wk, bk, Wv, bv, Wo, bo):
    """Host-side marshaling: slice per core, transpose tokens, cast bf16."""
    scale = 1.0 / np.sqrt(HEAD_DIM)
    per_batch_tokT = [
        np.ascontiguousarray(tokens[b].T).astype(BF16) for b in range(B)
    ]
    in_maps = []
    for core in range(N_CORES):
        b, g = divmod(core, GROUPS)
        cols = slice(g * DH, (g + 1) * DH)
        # weights as [128, 8*DH]: din-tile t at column block t
        wq_s = (Wq[:, cols] * scale).astype(BF16).reshape(8, 128, DH)
        wq_s = np.ascontiguousarray(wq_s.transpose(1, 0, 2)).reshape(128, 8 * DH)
        wk_s = Wk[:, cols].astype(BF16).reshape(8, 128, DH)
        wk_s = np.ascontiguousarray(wk_s.transpose(1, 0, 2)).reshape(128, 8 * DH)
        wv_s = Wv[:, cols].astype(BF16).reshape(8, 128, DH)
        wv_s = np.ascontiguousarray(wv_s.transpose(1, 0, 2)).reshape(128, 8 * DH)
        # Wo rows for this group, pair p at column block p
        wo_s = Wo[cols, :].astype(BF16).reshape(2, 128, D)
        wo_s = np.ascontiguousarray(wo_s.transpose(1, 0, 2)).reshape(128, 2 * D)
        # biases: column j = q-pair j (rows 0:64 even head, 64:128 odd),
        # column 2+j = k-pair j
        bqk_s = np.zeros((128, 4), np.float32)
        for j in range(2):
            bqk_s[0:64, j] = bq[g * DH + (2 * j) * 64:g * DH + (2 * j + 1) * 64] * scale
            bqk_s[64:128, j] = bq[g * DH + (2 * j + 1) * 64:g * DH + (2 * j + 2) * 64] * scale
            bqk_s[0:64, 2 + j] = bk[g * DH + (2 * j) * 64:g * DH + (2 * j + 1) * 64]
            bqk_s[64:128, 2 + j] = bk[g * DH + (2 * j + 1) * 64:g * DH + (2 * j + 2) * 64]
        in_maps.append({
            "tokT": per_batch_tokT[b],
            "wq": wq_s, "wk": wk_s, "wv": wv_s, "wo": wo_s,
            "bqk": bqk_s,
        })
    return in_maps


def kernel(tokens, Wq, bq, Wk, bk, Wv, bv, Wo, bo):
    tokens = np.asarray(tokens, np.float32)
    Wq = np.asarray(Wq, np.float32); bq = np.asarray(bq, np.float32)
    Wk = np.asarray(Wk, np.float32); bk = np.asarray(bk, np.float32)
    Wv = np.asarray(Wv, np.float32); bv = np.asarray(bv, np.float32)
    Wo = np.asarray(Wo, np.float32); bo = np.asarray(bo, np.float32)

    nc = _build()
    in_maps = _prep_core_inputs(tokens, Wq, bq, Wk, bk, Wv, bv, Wo, bo)
    res = bass_utils.run_bass_kernel_spmd(
        nc, in_maps, core_ids=list(range(N_CORES)))
    _CACHE["last_result"] = res

    bo_eff = (bv.astype(np.float64) @ Wo.astype(np.float64)
              + bo.astype(np.float64)).astype(np.float32)

    out = []
    for name in ("outT_neg", "outT_pos"):
        full = np.empty((B, S, D), np.float32)
        for b in range(B):
            acc = res.results[b * GROUPS][name].astype(np.float32)
            for g in range(1, GROUPS):
                acc += res.results[b * GROUPS + g][name].astype(np.float32)
            full[b] = acc.T
        full += bo_eff
        out.append(full)
    return tuple(out)


# revision 31
# speedup vs baseline: 1.0180x; 1.0113x over previous
"""Bidirectional attention (softmax(+logits) and softmax(-logits) branches)
on 8 Trainium2 NeuronCores.

Sharding: batch x head-group. Core c handles batch c//4 and heads
4*(c%4) .. 4*(c%4)+3. Each core computes its heads' Q/K/V projections,
both softmax branches, and a partial output projection (row-shard of Wo);
the host sums the 4 partials per batch and transposes.

All matmuls run in bf16 (fp32 matmul is 4x slower on the PE); PSUM
accumulation is fp32. The softmax uses unnormalized exp (logit range is
~N(0,1), no max-subtraction needed): P_pos = exp(+logits) via wide ACT
ops; P_neg = 1/P_pos via a single custom-DVE op (BITWISE_NOT seed + 2
inline Newton passes) run on the bf16 tiles directly -- this halves the
ACT engine's exp volume. Z comes free from a ones-column appended to V;
the 1/Z normalization is batched across both branches.

Pipeline structure (all HW-measured):
- ONE unified PSUM allocation: lgp (2x[128,1024]) + ftp (2x[128,1024])
  = 8 banks, live for the whole kernel.  Phases 1/3 borrow tiles from
  these rotations instead of opening their own pools (separate pools
  alias banks and serialize phase boundaries).  Matmul psum outputs
  must be BANK-aligned (mid-bank outputs corrupt on HW).
- Head-0 PRELUDE: the first head's logits/exp/recip are emitted before
  the V projection with its PV matmuls deferred, so the ACT exp stream
  starts as soon as Q/K are projected (~25us) instead of after the
  whole V projection (~50us).  The PV backlog drains 2-groups-per-
  iteration during head 1.
- PV matmuls trail their logits: pos by 2 groups, neg by 3.  The serial
  chains logits -> exp(ACT) -> PV-pos (~1.1us) and -> recip(DVE) ->
  PV-neg (~2.2us) would otherwise stall the in-order PE queue every
  group.  PSUM accumulation order within a region is free; only the
  start (st==0) / stop (st==SK-1) flags matter.
- The ACT-computed neg-branch exps sit at g==3/4 (mid-head), away from
  the head-boundary congestion.
- Output projection (phase 3) alternates its borrowed psum pair between
  the lg and ft rotations (4 tiles in flight) and splits each evac into
  concurrent DVE+ACT halves, so it runs matmul-paced.
- full-width head-pair projections: one P_out=128 matmul covers both
  heads of a pair; Q^T/K^T kept in base-0 and base-64 copies so
  consecutive K=64 logit matmuls alternate PE row-groups.

Host-side prep folds the 1/sqrt(d) scale into Wq, and bv@Wo+bo into a
host-side bias (exact because softmax rows sum to 1).
"""

import os
import sys

for _p in ("/opt/trn_rl_repo",):
    if _p not in sys.path:
        sys.path.insert(0, _p)

import numpy as np
import ml_dtypes

import concourse.bass as bass
import concourse.tile as tile
from concourse import bacc, mybir
from concourse import bass_utils
from concourse.dve_ops import RECIPROCAL_APPROX_FAST, RECIP_APPROX_FAST_CONSTS

BF16 = ml_dtypes.bfloat16

B, S, D = 2, 2048, 1024
NUM_HEADS, HEAD_DIM = 16, 64
N_CORES = 8
GROUPS = 4                      # head groups (one per core within a batch)
H = NUM_HEADS // GROUPS         # heads per core = 4
DH = H * HEAD_DIM               # per-core head dims = 256
QCH = 512                       # q-chunk (matmul moving free dim)
NCH = S // QCH                  # 4 q-chunks
SK = S // 128                   # 16 s_k tiles
VSTRIDE = H * 128               # V' row stride: 4 heads x (64 v + 64 ones)

f32 = mybir.dt.float32
bf16 = mybir.dt.bfloat16
EXP = mybir.ActivationFunctionType.Exp
IDENT = mybir.ActivationFunctionType.Identity

# How many groups the PV matmuls trail their group's logits.
POS_DELAY = 2
NEG_DELAY = 3
# Max deferred-PV groups flushed per ring iteration (drains the head-0
# prelude backlog without monopolizing the PE).
FLUSH_CAP = 3

_CACHE = {}


def _emit(tc):
    nc = tc.nc
    tokT = nc.dram_tensor("tokT", [D, S], bf16, kind="ExternalInput").ap()
    wq = nc.dram_tensor("wq", [128, 8 * DH], bf16, kind="ExternalInput").ap()
    wk = nc.dram_tensor("wk", [128, 8 * DH], bf16, kind="ExternalInput").ap()
    wv = nc.dram_tensor("wv", [128, 8 * DH], bf16, kind="ExternalInput").ap()
    wo = nc.dram_tensor("wo", [128, 2 * D], bf16, kind="ExternalInput").ap()
    bqk = nc.dram_tensor("bqk", [128, 4], f32, kind="ExternalInput").ap()
    outs = [
        nc.dram_tensor("outT_neg", [D, S], bf16, kind="ExternalOutput").ap(),
        nc.dram_tensor("outT_pos", [D, S], bf16, kind="ExternalOutput").ap(),
    ]

    rc = RECIP_APPROX_FAST_CONSTS

    import contextlib

    with contextlib.ExitStack() as ctx:
        wp = ctx.enter_context(tc.tile_pool(name="wp", bufs=1))
        act = ctx.enter_context(tc.tile_pool(name="act", bufs=1))
        ppool = ctx.enter_context(tc.tile_pool(name="pp", bufs=2))
        fpool = ctx.enter_context(tc.tile_pool(name="fp", bufs=4))
        zpool = ctx.enter_context(tc.tile_pool(name="zp", bufs=2))
        opool = ctx.enter_context(tc.tile_pool(name="op", bufs=8))
        # the single 8-bank PSUM allocation for the whole kernel
        lgp = ctx.enter_context(tc.tile_pool(name="lgp", bufs=2, space="PSUM"))
        ftp = ctx.enter_context(tc.tile_pool(name="ftp", bufs=2, space="PSUM"))

        # ---- weight / bias / token loads -------------------------------
        # input loads in COMPUTE order: the K projection's t-chain chases
        # tok tiles arriving on sync, while the other weights ride the
        # gpsimd/scalar queues (HBM bandwidth is the real floor, so the
        # first-needed data must not share it with late-needed data)
        wq_sb = wp.tile([128, 8 * DH], bf16, tag="wq")
        wk_sb = wp.tile([128, 8 * DH], bf16, tag="wk")
        wv_sb = wp.tile([128, 8 * DH], bf16, tag="wv")
        wo_sb = wp.tile([128, 2 * D], bf16, tag="wo")
        bqk_sb = wp.tile([128, 4], f32, tag="bqk")
        # everything rides ONE queue in strict need-order so the
        # first-needed bytes get the full HBM bandwidth: wk+wq (the
        # concurrent K/Q t-chains), tok (their pacer), wv (V starts
        # after the Q evacs), wo (needed only at phase 3)
        nc.sync.dma_start(wk_sb[:], wk)
        nc.sync.dma_start(wq_sb[:], wq)
        nc.gpsimd.dma_start(bqk_sb[:], bqk)

        tok = []
        for t in range(8):
            tt = act.tile([128, S], bf16, tag=f"tok{t}")
            nc.sync.dma_start(tt[:], tokT[t * 128:(t + 1) * 128, :])
            tok.append(tt)
        nc.sync.dma_start(wv_sb[:], wv)
        nc.sync.dma_start(wo_sb[:], wo)

        # head-pair tiles: even head in partitions 0:64, odd in 64:128;
        # *_alt has the two halves swapped (so every head exists at both
        # partition bases -- lets logit matmuls alternate PE row groups)
        qt_pair = [act.tile([128, S], bf16, tag=f"qp{j}", name=f"qp{j}") for j in range(2)]
        kt_pair = [act.tile([128, S], bf16, tag=f"kp{j}", name=f"kp{j}") for j in range(2)]
        qt_alt = [act.tile([128, S], bf16, tag=f"qa{j}", name=f"qa{j}") for j in range(2)]
        kt_alt = [act.tile([128, S], bf16, tag=f"ka{j}", name=f"ka{j}") for j in range(2)]
        SKQ = SK // 4
        vp_q = [act.tile([128, SKQ * VSTRIDE], bf16, tag=f"vp{i}", name=f"vp{i}")
                for i in range(4)]

        def vap_for(st, h):
            q, r = divmod(st, SKQ)
            off = r * VSTRIDE + h * 128
            return vp_q[q][:, off:off + 128]

        # V' head block is [ones (64) | v (64)]: the 64 ones-columns make
        # the PV matmul write Z replicated across psum rows 0:64 -- a free
        # partition-broadcast of the softmax denominator (and a full
        # 128-col stationary enables FWL).  Z sits at base partition 0
        # because the custom-DVE reciprocal only works at base 0
        # (HW-verified); feat lands at rows 64:128.
        for q in range(4):
            ones_ap = vp_q[q].rearrange(
                "p (s h x) -> p (s h) x", s=SKQ, h=H)[:, :, 0:64]
            nc.gpsimd.memset(ones_ap, 1.0)

        def lgp_chains():
            tiles = [lgp.tile([128, GW * QCH], f32, tag="lg",
                              name=f"pjb{x}") for x in range(2)]
            return [tiles[x // 2][:, (x % 2) * QCH:(x % 2 + 1) * QCH]
                    for x in range(4)]

        def ftp_chains():
            tiles = [ftp.tile([128, GW * QCH], f32, tag="ft",
                              name="ft") for x in range(2)]
            return [tiles[x // 2][:, (x % 2) * QCH:(x % 2 + 1) * QCH]
                    for x in range(4)]

        # ---- phase 1: projections (borrowed ring psum tiles) -----------
        # (matmul psum outputs are capped at one bank = 512 f32 columns,
        # and must be bank-aligned)
        def qk_evac(j, pair, alt, ps, bcol0, on_dve):
            # evacs split across ACT (identity+bias) and DVE
            # (tensor_scalar add) so 8 of them don't serialize one engine.
            # The alt copies go out PER-CHUNK right behind each evac so
            # the first ring logits become READY as soon as chunk 0 is
            # projected -- the list scheduler orders by readiness, and an
            # all-chunks alt gate let the pair-1/V bulk win the early PE
            # slots (delaying every exp via the PE counter semaphore).
            for cc in range(4):
                cs = slice(cc * QCH, (cc + 1) * QCH)
                bias_ap = bqk_sb[:, bcol0 + j:bcol0 + j + 1]
                if on_dve:
                    nc.vector.tensor_scalar_add(pair[j][:, cs], ps[cc], bias_ap)
                else:
                    nc.scalar.activation(pair[j][:, cs], ps[cc], IDENT,
                                         bias=bias_ap)
                nc.sync.dma_start(alt[j][64:128, cs], pair[j][0:64, cs])
                nc.sync.dma_start(alt[j][0:64, cs], pair[j][64:128, cs])

        def qk_chain_t(ps, w_sb, base, t, first, last):
            for cc in range(4):
                nc.tensor.matmul(ps[cc], w_sb[:, base:base + 128],
                                 tok[t][:, cc * QCH:(cc + 1) * QCH],
                                 start=first, stop=last)

        def qk_proj(j):
            # full-width head-pair matmuls: one P_out=128 matmul per
            # (din-tile, chunk-pair) covers both heads (even head -> psum
            # rows 0:64, odd -> 64:128 -- weight cols are adjacent).
            # Pair 0 runs K and Q as CONCURRENT t-chains (K on the lg
            # psum pair, Q on the ft pair) so both finish with the token
            # DMA instead of Q serializing behind K's evac.  Pair 1 runs
            # mid-ring where the ft rotation is busy, so its Q chains
            # reuse the lg pair sequentially.
            base_j = j * 128
            ps_k = lgp_chains()
            ps_q = ftp_chains()
            for t in range(8):
                base = t * DH + base_j
                qk_chain_t(ps_k, wk_sb, base, t, t == 0, t == 7)
                qk_chain_t(ps_q, wq_sb, base, t, t == 0, t == 7)
            qk_evac(j, kt_pair, kt_alt, ps_k, 2, on_dve=False)
            qk_evac(j, qt_pair, qt_alt, ps_q, 0, on_dve=True)

        def emit_vproj():
            # V: 2 s-tile chains per sp-group, carved from one borrowed
            # ftp tile at BANK-ALIGNED offsets (0 and QCH*4B=2KB).
            # Matmul psum outputs at mid-bank offsets corrupt on HW, so
            # each [128,256] chain gets its own bank (half unused).
            for sp in range(SK // 2):
                ftv = ftp.tile([128, 2 * QCH], f32, tag="ft", name="ft")
                psv = [ftv[:, i * QCH:i * QCH + DH] for i in range(2)]
                for t in range(8):
                    for i in range(2):
                        st = sp * 2 + i
                        nc.tensor.matmul(
                            psv[i],
                            tok[t][:, st * 128:(st + 1) * 128],
                            wv_sb[:, t * DH:(t + 1) * DH],
                            start=(t == 0), stop=(t == 7),
                        )
                for i in range(2):
                    st = sp * 2 + i
                    q, r = divmod(st, SKQ)
                    dst = vp_q[q][:, r * VSTRIDE:(r + 1) * VSTRIDE]
                    dst3 = dst.rearrange("p (h x) -> p h x", h=H)[:, :, 64:128]
                    src3 = psv[i].rearrange("p (h x) -> p h x", h=H)
                    nc.vector.tensor_copy(dst3, src3)

        GW = 2           # s_k-tiles per exp group (FD = GW*QCH = 1024)
        NG = SK // GW    # 8 groups per (head, chunk)

        qk_proj(0)
        # (pair-1 q/k and the V projection are emitted at (c0,h1,g0),
        # AFTER head 0's logits/exp prelude: emission order is the
        # scheduler's tie-break, so the prelude's logits win the early
        # PE slots and the exp stream starts as soon as chunk-0 q/k are
        # projected; the PE absorbs the deferred bulk underneath it)

        # ---- phase 2: attention ----------------------------------------
        fsb_all = []
        # phase-3 output DMAs rotate over the three DMA-capable queues
        out_engines = [nc.sync, nc.gpsimd, nc.scalar]
        u_out = [0]

        # per-(c,h) feature psum, allocated lazily at the first deferred
        # PV flush so the ft rotation order matches actual use
        ft_map = {}

        def get_ft(c2, h2):
            if (c2, h2) not in ft_map:
                ft_map[(c2, h2)] = ftp.tile([128, 2 * QCH], f32, tag="ft",
                                            name="ft")
            return ft_map[(c2, h2)]

        def outproj_dp(c, dp):
            # one dout-pair iteration of the output projection: a psum
            # pair borrowed alternately from the lg / ft rotations (so 4
            # tiles pipeline), 2 dh-passes x 2 branches accumulation,
            # evac split into concurrent DVE (neg) + ACT (pos) halves,
            # output DMAs on sync/gpsimd
            pool, tg = (lgp, "lg") if dp % 2 == 0 else (ftp, "ft")
            ops = [pool.tile([128, GW * QCH], f32, tag=tg, name=f"ops{i}")
                   for i in range(2)]
            for p in range(2):
                for i in range(2):
                    dt = dp * 2 + i
                    lhs = wo_sb[:, p * D + dt * 128:p * D + (dt + 1) * 128]
                    for br in range(2):
                        nc.tensor.matmul(
                            ops[i][:, br * QCH:(br + 1) * QCH],
                            lhs,
                            fsb_all[c][p][:, br * QCH:(br + 1) * QCH],
                            start=(p == 0), stop=(p == 1),
                        )
            for i in range(2):
                dt = dp * 2 + i
                osb = opool.tile([128, 2 * QCH], bf16, tag="os", name="osb")
                nc.vector.tensor_copy(osb[:, 0:QCH], ops[i][:, 0:QCH])
                nc.scalar.copy(osb[:, QCH:2 * QCH], ops[i][:, QCH:2 * QCH])
                for br in range(2):
                    eng = out_engines[u_out[0] % len(out_engines)]
                    u_out[0] += 1
                    eng.dma_start(
                        outs[br][dt * 128:(dt + 1) * 128,
                                 c * QCH:(c + 1) * QCH],
                        osb[:, br * QCH:(br + 1) * QCH],
                    )

        # deferred normalization: the recip -> zhi-DMA -> mul chain of
        # head h is emitted split across head h+1's groups 3/5 (after
        # head h's delayed PV matmuls have all been flushed), so the
        # sync-DMA round trip overlaps queued recip work instead of
        # stalling the strict-FIFO DVE queue at each head boundary
        def norm_start(stt):
            c2, h2, jp, parp, fsbp = stt
            ft2p = ft_map[(c2, h2)]
            zb = zpool.tile([64, 2 * QCH], f32, tag="zb", name="zb")
            zhi = zpool.tile([128, 2 * QCH], f32, tag="zhi", name="zhi")
            nc.vector.reciprocal_approx_fast(zb[:], ft2p[0:64, :])
            nc.sync.dma_start(zhi[64:128, :], zb[:])
            return (ft2p, parp, fsbp, zhi)

        def norm_finish(stt):
            ft2p, parp, fsbp, zhi = stt
            if parp == 1:
                # odd head: feat rows 64:128 align with the fsb
                # destination -- write it directly
                nc.vector.tensor_mul(fsbp[64:128, :],
                                     ft2p[64:128, :], zhi[64:128, :])
            else:
                tmp = zpool.tile([128, 2 * QCH], bf16, tag="tmp",
                                 name="tmp")
                nc.vector.tensor_mul(tmp[64:128, :], ft2p[64:128, :],
                                     zhi[64:128, :])
                nc.sync.dma_start(fsbp[0:64, :], tmp[64:128, :])

        def emit_logits(c2, h2, g2):
            # K=64 logit matmul pair on alternating PE row groups
            j2, par2 = h2 // 2, h2 % 2
            k_lo = kt_pair[j2] if par2 == 0 else kt_alt[j2]
            k_hi = kt_alt[j2] if par2 == 0 else kt_pair[j2]
            q_lo = qt_pair[j2] if par2 == 0 else qt_alt[j2]
            q_hi = qt_alt[j2] if par2 == 0 else qt_pair[j2]
            lg = lgp.tile([128, GW * QCH], f32, tag="lg")
            for t2 in range(GW):
                st = g2 * GW + t2
                if st % 2 == 0:
                    nc.tensor.matmul(
                        lg[:, t2 * QCH:(t2 + 1) * QCH],
                        k_lo[0:64, st * 128:(st + 1) * 128],
                        q_lo[0:64, c2 * QCH:(c2 + 1) * QCH],
                        start=True, stop=True)
                else:
                    nc.tensor.matmul(
                        lg[:, t2 * QCH:(t2 + 1) * QCH],
                        k_hi[64:128, st * 128:(st + 1) * 128],
                        q_hi[64:128, c2 * QCH:(c2 + 1) * QCH],
                        start=True, stop=True)
            return lg

        def emit_pv(item, br):
            pw, c2, h2, g2 = item
            ft2_ = get_ft(c2, h2)
            for t2 in range(GW):
                st = g2 * GW + t2
                nc.tensor.matmul(
                    ft2_[:, br * QCH:(br + 1) * QCH],
                    vap_for(st, h2),
                    pw[:, t2 * QCH:(t2 + 1) * QCH],
                    start=(st == 0),
                    stop=(st == SK - 1),
                )

        # flattened group sequence.  Logits are software-pipelined ONE
        # group ahead; PV-pos trails POS_DELAY and PV-neg NEG_DELAY
        # groups so the ACT/DVE chains are covered by other PE work.
        # Head (c0,h0) is a PRELUDE: its PV is fully deferred past the
        # V projection.
        groups = [(c, h, g) for c in range(NCH) for h in range(H)
                  for g in range(NG)]
        lg_next = None
        pend = pend2 = None
        pend_pos = []
        pend_neg = []

        for i, (c, h, g) in enumerate(groups):
            j, par = h // 2, h % 2
            if g == 0 and h == 0:
                fsb = [fpool.tile([128, 2 * QCH], bf16, tag="fsb",
                                  bufs=8, name=f"fsb{j2}")
                       for j2 in range(2)]
                fsb_all.append(fsb)
                fsb_c = fsb
            if c == 0 and h == 1 and g == 0:
                # the deferred phase-1 bulk (pair-1 q/k, then the V
                # projection), emitted under head 0's already-running
                # exp/recip stream
                qk_proj(1)
                emit_vproj()
            if i == 0:
                lg_next = emit_logits(c, h, g)
            lg = lg_next
            if i + 1 < len(groups):
                lg_next = emit_logits(*groups[i + 1])
            pw_pos = ppool.tile([128, GW * QCH], bf16, tag="pwp",
                                name="pwp", bufs=11)
            pw_neg = ppool.tile([128, GW * QCH], bf16, tag="pwn",
                                name="pwn", bufs=12)
            nc.scalar.activation(pw_pos[:], lg[:], EXP)
            if g == 3 or (g == 4 and ((h + c) & 1)):
                # mid-head ACT-exp groups balance ACT vs DVE without
                # piling a second exp onto the busy head-boundary groups
                nc.scalar.activation(pw_neg[:], lg[:], EXP, scale=-1.0)
            else:
                # P_neg = 1/P_pos on DVE (bf16 in/out; the DVE read
                # path converts bf16->fp32 bit-exactly so the fp32
                # BITWISE_NOT seed applies unchanged)
                nc.vector._custom_dve(
                    RECIPROCAL_APPROX_FAST,
                    out=pw_neg[:], in0=pw_pos[:],
                    s0=rc["s0"], s1=rc["s1"], imm2=rc["imm2"],
                )
            if not (c == 0 and h == 0):
                # flush deferred PV: neg first (its recip is oldest)
                for _ in range(FLUSH_CAP):
                    if len(pend_neg) > NEG_DELAY:
                        emit_pv(pend_neg.pop(0), 0)
                for _ in range(FLUSH_CAP):
                    if len(pend_pos) > POS_DELAY:
                        emit_pv(pend_pos.pop(0), 1)
            pend_pos.append((pw_pos, c, h, g))
            pend_neg.append((pw_neg, c, h, g))
            # previous head's deferred normalization: feat / Z.  Z sits
            # pre-broadcast in its psum rows 0:64 (the ones-columns),
            # feat in rows 64:128, both branches in one [64, 2*QCH] op.
            # recip at base 0 (the custom-DVE op requires it), one SBUF
            # DMA shifts 1/Z up to rows 64:128, multiply straight out of
            # PSUM.
            if g == 3 and pend is not None:
                pend2, pend = norm_start(pend), None
            elif g == 5 and pend2 is not None:
                norm_finish(pend2)
                pend2 = None
            if g == NG - 1:
                pend = (c, h, j, par, fsb_c[j])

        while pend_pos or pend_neg:
            if pend_neg:
                emit_pv(pend_neg.pop(0), 0)
            if pend_pos:
                emit_pv(pend_pos.pop(0), 1)
        norm_finish(norm_start(pend))

        # ---- phase 3: output projection (borrowed lg/ft psum) ----------
        for c in range(NCH):
            for dp in range(4):
                outproj_dp(c, dp)


def _build():
    if "nc" in _CACHE:
        return _CACHE["nc"]
    nc = bacc.Bacc("TRN2", target_bir_lowering=False, debug=False,
                   num_devices=N_CORES)
    with tile.TileContext(nc) as tc:
        _emit(tc)
    nc.compile()
    _CACHE["nc"] = nc
    return nc


def _prep_core_inputs(tokens, Wq, bq, Wk, bk, Wv, bv, Wo, bo):
    """Host-side marshaling: slice per core, transpose tokens, cast bf16."""
    scale = 1.0 / np.sqrt(HEAD_DIM)
    per_batch_tokT = [
        np.ascontiguousarray(tokens[b].T).astype(BF16) for b in range(B)
    ]
    in_maps = []
    for core in range(N_CORES):
        b, g = divmod(core, GROUPS)
        cols = slice(g * DH, (g + 1) * DH)
        # weights as [128, 8*DH]: din-tile t at column block t
        wq_s = (Wq[:, cols] * scale).astype(BF16).reshape(8, 128, DH)
        wq_s = np.ascontiguousarray(wq_s.transpose(1, 0, 2)).reshape(128, 8 * DH)
        wk_s = Wk[:, cols].astype(BF16).reshape(8, 128, DH)
        wk_s = np.ascontiguousarray(wk_s.transpose(1, 0, 2)).reshape(128, 8 * DH)
        wv_s = Wv[:, cols].astype(BF16).reshape(8, 128, DH)
        wv_s = np.ascontiguousarray(wv_s.transpose(1, 0, 2)).reshape(128, 8 * DH)
        # Wo rows for this group, pair p at column block p
        wo_s = Wo[cols, :].astype(BF16).reshape(2, 128, D)
        wo_s = np.ascontiguousarray(wo_s.transpose(1, 0, 2)).reshape(128, 2 * D)
        # biases: column j = q-pair j (rows 0:64 even head, 64:128 odd),
        # column 2+j = k-pair j
        bqk_s = np.zeros((128, 4), np.float32)
        for j in range(2):
            bqk_s[0:64, j] = bq[g * DH + (2 * j) * 64:g * DH + (2 * j + 1) * 64] * scale
            bqk_s[64:128, j] = bq[g * DH + (2 * j + 1) * 64:g * DH + (2 * j + 2) * 64] * scale
            bqk_s[0:64, 2 + j] = bk[g * DH + (2 * j) * 64:g * DH + (2 * j + 1) * 64]
            bqk_s[64:128, 2 + j] = bk[g * DH + (2 * j + 1) * 64:g * DH + (2 * j + 2) * 64]
        in_maps.append({
            "tokT": per_batch_tokT[b],
            "wq": wq_s, "wk": wk_s, "wv": wv_s, "wo": wo_s,
            "bqk": bqk_s,
        })
    return in_maps


def kernel(tokens, Wq, bq, Wk, bk, Wv, bv, Wo, bo):
    tokens = np.asarray(tokens, np.float32)
    Wq = np.asarray(Wq, np.float32); bq = np.asarray(bq, np.float32)
    Wk = np.asarray(Wk, np.float32); bk = np.asarray(bk, np.float32)
    Wv = np.asarray(Wv, np.float32); bv = np.asarray(bv, np.float32)
    Wo = np.asarray(Wo, np.float32); bo = np.asarray(bo, np.float32)

    nc = _build()
    in_maps = _prep_core_inputs(tokens, Wq, bq, Wk, bk, Wv, bv, Wo, bo)
    res = bass_utils.run_bass_kernel_spmd(
        nc, in_maps, core_ids=list(range(N_CORES)))
    _CACHE["last_result"] = res

    bo_eff = (bv.astype(np.float64) @ Wo.astype(np.float64)
              + bo.astype(np.float64)).astype(np.float32)

    out = []
    for name in ("outT_neg", "outT_pos"):
        full = np.empty((B, S, D), np.float32)
        for b in range(B):
            acc = res.results[b * GROUPS][name].astype(np.float32)
            for g in range(1, GROUPS):
                acc += res.results[b * GROUPS + g][name].astype(np.float32)
            full[b] = acc.T
        full += bo_eff
        out.append(full)
    return tuple(out)
